# revision 7
# baseline (speedup 1.0000x reference)
"""Trainium2 Bass kernel: Mixtral-style MoE block (8 experts, top-2 router).

Sharding: expert-parallel across 8 NeuronCores — core c owns expert c's
w_gate/w_inter/w_out. The router is replicated (every core computes logits +
top-2 weights and scales its own expert's output by its routing weight);
the weighted expert sum is realized as a host-side sum over the 8 per-core
partial outputs at gather time.

Per-core compute layout (all matmuls via PE in float32r = full rate):
  flatT [H, T] (host-pretransposed)  ->  gateT/interT tiles [I_tile, T]
  hiddenT = silu(gateT) * interT     ->  out [T, H] = hiddenT.T @ w_out
The second matmul uses hiddenT tiles directly as the stationary operand, so
no on-device transposes are needed anywhere.
"""

import numpy as np

# ---- problem constants (hardcoded per contract) ----
P = 128
T = 4096           # tokens (B*S = 2*2048)
H = 1024           # hidden
ID = 2048          # intermediate
E = 8              # experts
NH = H // P        # 8 h-tiles
NI = ID // P       # 16 i-tiles
NB = 4             # token blocks
TB = T // NB       # 1024 tokens per block
NT = TB // P       # 8 t-tiles per block
NC2 = 512          # matmul moving free-dim chunk
NCORES = 8

_CACHE = {}


def _build_nc():
    import concourse.bass as bass
    import concourse.bacc as bacc
    from concourse import mybir
    from concourse import tile

    F32 = mybir.dt.float32
    F32R = mybir.dt.float32r
    AF = mybir.ActivationFunctionType
    OP = mybir.AluOpType
    AX = mybir.AxisListType

    nc = bacc.Bacc(None, target_bir_lowering=False)

    # DRAM I/O (host-pretiled layouts; see kernel() below)
    flatT = nc.dram_tensor("flatT", [NH, P, T], F32R, kind="ExternalInput")
    rk = nc.dram_tensor("rk", [NH, P, E], F32, kind="ExternalInput")
    wg = nc.dram_tensor("wg", [NI, P, NH * P], F32R, kind="ExternalInput")
    wi = nc.dram_tensor("wi", [NI, P, NH * P], F32R, kind="ExternalInput")
    wo = nc.dram_tensor("wo", [NI, P, H], F32R, kind="ExternalInput")
    sel = nc.dram_tensor("sel", [P, E], F32, kind="ExternalInput")
    out = nc.dram_tensor("out", [T, H], F32, kind="ExternalOutput")
    logits_out = nc.dram_tensor("logits", [T, E], F32, kind="ExternalOutput")

    with tile.TileContext(nc) as tc:
        with tc.tile_pool(name="consts", bufs=1) as consts, \
             tc.tile_pool(name="flatp", bufs=9) as flatp, \
             tc.tile_pool(name="wgp", bufs=3) as wgp, \
             tc.tile_pool(name="wip", bufs=3) as wip, \
             tc.tile_pool(name="wop", bufs=10) as wop, \
             tc.tile_pool(name="hidp", bufs=9) as hidp, \
             tc.tile_pool(name="evp", bufs=4) as evp, \
             tc.tile_pool(name="accp", bufs=17) as accp, \
             tc.tile_pool(name="outp", bufs=4) as outp, \
             tc.tile_pool(name="rtp", bufs=4) as rtp, \
             tc.tile_pool(name="psg", bufs=2, space="PSUM") as psg, \
             tc.tile_pool(name="psv", bufs=2, space="PSUM") as psv, \
             tc.tile_pool(name="pso", bufs=2, space="PSUM") as pso, \
             tc.tile_pool(name="psr", bufs=2, space="PSUM") as psr:

            # ---- constants ----
            rk_sb = consts.tile([P, NH, E], F32)
            nc.sync.dma_start(out=rk_sb[:], in_=rk[:].rearrange("h p e -> p h e"))
            sel_sb = consts.tile([P, E], F32)
            nc.sync.dma_start(out=sel_sb[:], in_=sel[:])
            # per-token routing scalar for this core's expert, one column per
            # global t-tile
            rsc = consts.tile([P, NB * NT], F32)

            for blk in range(NB):
                t0 = blk * TB

                # ---- load this block's activation tiles [P(h), TB] ----
                ft = []
                for hh in range(NH):
                    f_t = flatp.tile([P, TB], F32R, name="ft")
                    nc.sync.dma_start(out=f_t[:], in_=flatT[hh, :, t0:t0 + TB])
                    ft.append(f_t)

                # ---- router for this block (true fp32 matmuls; tiny) ----
                for tt in range(NT):
                    g_tt = blk * NT + tt
                    ps_r = psr.tile([P, E], F32, name="ps_r")
                    for hh in range(NH):
                        nc.tensor.matmul(
                            ps_r[:],
                            lhsT=ft[hh][:, tt * P:(tt + 1) * P].bitcast(F32),
                            rhs=rk_sb[:, hh, :],
                            start=(hh == 0),
                            stop=(hh == NH - 1),
                        )
                    L = rtp.tile([P, E], F32, name="L")
                    nc.vector.tensor_copy(L[:], ps_r[:])
                    nc.sync.dma_start(
                        out=logits_out[t0 + tt * P: t0 + (tt + 1) * P, :],
                        in_=L[:],
                    )
                    # top-2 + renormalize:  w1 = sigmoid(l1-l2), w2 = 1-w1
                    m1 = rtp.tile([P, 1], F32, name="m1")
                    nc.vector.tensor_reduce(m1[:], L[:], axis=AX.X, op=OP.max)
                    is1 = rtp.tile([P, E], F32, name="is1")
                    nc.vector.tensor_scalar(
                        out=is1[:], in0=L[:], scalar1=m1[:], scalar2=None,
                        op0=OP.is_ge,
                    )
                    big = rtp.tile([P, E], F32, name="big")
                    nc.vector.tensor_scalar_mul(big[:], is1[:], -1e30)
                    Lm = rtp.tile([P, E], F32, name="Lm")
                    nc.vector.tensor_add(Lm[:], L[:], big[:])
                    m2 = rtp.tile([P, 1], F32, name="m2")
                    nc.vector.tensor_reduce(m2[:], Lm[:], axis=AX.X, op=OP.max)
                    d12 = rtp.tile([P, 1], F32, name="d12")
                    nc.vector.tensor_sub(d12[:], m1[:], m2[:])
                    s1 = rtp.tile([P, 1], F32, name="s1")
                    nc.scalar.activation(s1[:], d12[:], AF.Sigmoid)
                    s2 = rtp.tile([P, 1], F32, name="s2")
                    nc.vector.tensor_scalar(
                        out=s2[:], in0=s1[:], scalar1=-1.0, scalar2=1.0,
                        op0=OP.mult, op1=OP.add,
                    )
                    sel2 = rtp.tile([P, E], F32, name="sel2")
                    nc.vector.tensor_scalar(
                        out=sel2[:], in0=L[:], scalar1=m2[:], scalar2=None,
                        op0=OP.is_ge,
                    )
                    is2 = rtp.tile([P, E], F32, name="is2")
                    nc.vector.tensor_sub(is2[:], sel2[:], is1[:])
                    w1 = rtp.tile([P, E], F32, name="w1")
                    nc.vector.tensor_scalar_mul(w1[:], is1[:], s1[:])
                    wr = rtp.tile([P, E], F32, name="wr")
                    nc.vector.scalar_tensor_tensor(
                        out=wr[:], in0=is2[:], scalar=s2[:], in1=w1[:],
                        op0=OP.mult, op1=OP.add,
                    )
                    wsel = rtp.tile([P, E], F32, name="wsel")
                    nc.vector.tensor_mul(wsel[:], wr[:], sel_sb[:])
                    nc.vector.tensor_reduce(
                        rsc[:, g_tt:g_tt + 1], wsel[:], axis=AX.X, op=OP.add,
                    )

                # ---- two half-sweeps over I: SwiGLU then down-proj partial ----
                acc = {}
                for half in range(2):
                    i_lo, i_hi = half * (NI // 2), (half + 1) * (NI // 2)

                    # phase 1: hiddenT tiles [P(i), TB] for this half
                    hid = {}
                    for ii in range(i_lo, i_hi):
                        wg_t = wgp.tile([P, NH * P], F32R, name="wg_t")
                        nc.sync.dma_start(out=wg_t[:], in_=wg[ii, :, :])
                        wi_t = wip.tile([P, NH * P], F32R, name="wi_t")
                        nc.sync.dma_start(out=wi_t[:], in_=wi[ii, :, :])
                        hid_t = hidp.tile([P, TB], F32R, name="hid_t")
                        for tcx in range(TB // NC2):
                            sl = slice(tcx * NC2, (tcx + 1) * NC2)
                            ps_g = psg.tile([P, NC2], F32, name="ps_g")
                            ps_v = psv.tile([P, NC2], F32, name="ps_v")
                            for hh in range(NH):
                                nc.tensor.matmul(
                                    ps_g[:],
                                    lhsT=wg_t[:, hh * P:(hh + 1) * P],
                                    rhs=ft[hh][:, sl],
                                    start=(hh == 0), stop=(hh == NH - 1),
                                )
                            for hh in range(NH):
                                nc.tensor.matmul(
                                    ps_v[:],
                                    lhsT=wi_t[:, hh * P:(hh + 1) * P],
                                    rhs=ft[hh][:, sl],
                                    start=(hh == 0), stop=(hh == NH - 1),
                                )
                            si_t = evp.tile([P, NC2], F32, name="si_t")
                            nc.scalar.activation(si_t[:], ps_g[:], AF.Silu)
                            nc.vector.tensor_mul(hid_t[:, sl], si_t[:], ps_v[:])
                        hid[ii] = hid_t

                    # phase 2: out[t,h] partial over this I-half
                    for ii in range(i_lo, i_hi):
                        wo_t = wop.tile([P, H], F32R, name="wo_t")
                        nc.sync.dma_start(out=wo_t[:], in_=wo[ii, :, :])
                        hid[ii] = (hid[ii], wo_t)

                    for tt in range(NT):
                        g_tt = blk * NT + tt
                        tsl = slice(tt * P, (tt + 1) * P)
                        for hcx in range(H // NC2):
                            hsl = slice(hcx * NC2, (hcx + 1) * NC2)
                            ps_o = pso.tile([P, NC2], F32, name="ps_o")
                            for ii in range(i_lo, i_hi):
                                hid_t, wo_t = hid[ii]
                                nc.tensor.matmul(
                                    ps_o[:],
                                    lhsT=hid_t[:, tsl],
                                    rhs=wo_t[:, hsl],
                                    start=(ii == i_lo), stop=(ii == i_hi - 1),
                                )
                            if half == 0:
                                # stash routing-scaled partial in SBUF
                                a_t = accp.tile([P, NC2], F32, name="a_t")
                                nc.vector.tensor_scalar_mul(
                                    a_t[:], ps_o[:], rsc[:, g_tt:g_tt + 1],
                                )
                                acc[(tt, hcx)] = a_t
                            else:
                                o_t = outp.tile([P, NC2], F32, name="o_t")
                                nc.vector.scalar_tensor_tensor(
                                    out=o_t[:], in0=ps_o[:],
                                    scalar=rsc[:, g_tt:g_tt + 1],
                                    in1=acc[(tt, hcx)][:],
                                    op0=OP.mult, op1=OP.add,
                                )
                                nc.sync.dma_start(
                                    out=out[t0 + tt * P: t0 + (tt + 1) * P, hsl],
                                    in_=o_t[:],
                                )
    nc.finalize()
    return nc


def _prep_in_maps(hidden_states, router_kernel, w_gate, w_inter, w_out):
    f32 = np.float32
    flat = np.ascontiguousarray(hidden_states, dtype=f32).reshape(T, H)
    flatT = np.ascontiguousarray(flat.T).reshape(NH, P, T)
    rk = np.ascontiguousarray(router_kernel, dtype=f32).reshape(NH, P, E)
    in_maps = []
    for c in range(NCORES):
        wg_c = np.ascontiguousarray(
            np.asarray(w_gate[c], dtype=f32).reshape(NH, P, NI, P)
            .transpose(2, 1, 0, 3).reshape(NI, P, NH * P))
        wi_c = np.ascontiguousarray(
            np.asarray(w_inter[c], dtype=f32).reshape(NH, P, NI, P)
            .transpose(2, 1, 0, 3).reshape(NI, P, NH * P))
        wo_c = np.ascontiguousarray(
            np.asarray(w_out[c], dtype=f32).reshape(NI, P, H))
        sel_c = np.zeros((P, E), dtype=f32)
        sel_c[:, c] = 1.0
        in_maps.append({
            "flatT": flatT, "rk": rk,
            "wg": wg_c, "wi": wi_c, "wo": wo_c, "sel": sel_c,
        })
    return in_maps


def run_on_device(hidden_states, router_kernel, w_gate, w_inter, w_out,
                  trace=False, **trace_kw):
    """Build (cached), run SPMD on 8 cores; returns (out, logits, results)."""
    from concourse.bass_utils import run_bass_kernel_spmd

    if "nc" not in _CACHE:
        _CACHE["nc"] = _build_nc()
    nc = _CACHE["nc"]

    in_maps = _prep_in_maps(
        hidden_states, router_kernel, w_gate, w_inter, w_out)
    res = run_bass_kernel_spmd(
        nc, in_maps, core_ids=list(range(NCORES)), trace=trace, **trace_kw)
    parts = res.results
    out = parts[0]["out"].astype(np.float32, copy=True)
    for c in range(1, NCORES):
        out += parts[c]["out"]
    logits = parts[0]["logits"]
    return out.reshape(2, 2048, H), logits, res


def kernel(hidden_states, router_kernel, w_gate, w_inter, w_out):
    out, logits, _ = run_on_device(
        hidden_states, router_kernel, w_gate, w_inter, w_out)
    return out, logits


# revision 14
# speedup vs baseline: 12632.5914x; 12632.5914x over previous
"""Trainium2 Bass kernel: Mixtral-style MoE block (8 experts, top-2 router).

Sharding: expert-parallel across 8 NeuronCores — core c owns expert c's
w_gate/w_inter/w_out. The router is replicated (every core computes logits +
top-2 weights and scales its own expert's output by its routing weight);
the weighted expert sum is realized as a host-side sum over the 8 per-core
partial outputs at gather time.

Per-core compute layout (all matmuls via PE in float32r = full rate):
  flatT [H, T] (host-pretransposed)  ->  gateT/interT tiles [I_tile, T]
  hiddenT = silu(gateT) * interT     ->  out [T, H] = hiddenT.T @ w_out
The second matmul uses hiddenT tiles directly as the stationary operand, so
no on-device transposes are needed anywhere.
"""

import numpy as np

# ---- problem constants (hardcoded per contract) ----
P = 128
T = 4096           # tokens (B*S = 2*2048)
H = 1024           # hidden
ID = 2048          # intermediate
E = 8              # experts
NH = H // P        # 8 h-tiles
NI = ID // P       # 16 i-tiles
NB = 4             # token blocks
TB = T // NB       # 1024 tokens per block
NT = TB // P       # 8 t-tiles per block
NC2 = 512          # matmul moving free-dim chunk
NCORES = 8

_CACHE = {}


def _build_nc():
    import concourse.bass as bass
    import concourse.bacc as bacc
    from concourse import mybir
    from concourse import tile
    from concourse import masks

    F32 = mybir.dt.float32
    F32R = mybir.dt.float32r
    AF = mybir.ActivationFunctionType
    OP = mybir.AluOpType
    AX = mybir.AxisListType

    nc = bacc.Bacc(None, target_bir_lowering=False)

    # DRAM I/O (host-pretiled layouts; see kernel() below)
    # flatT carries the FP22-truncated "hi" part; flatT_lo the residual, so
    # the router can form an exactly-compensated fp32 logit product.
    flatT = nc.dram_tensor("flatT", [NH, P, T], F32R, kind="ExternalInput")
    flatT_lo = nc.dram_tensor("flatT_lo", [NH, P, T], F32R, kind="ExternalInput")
    rk_h = nc.dram_tensor("rk_h", [NH, P, E], F32R, kind="ExternalInput")
    rk_l = nc.dram_tensor("rk_l", [NH, P, E], F32R, kind="ExternalInput")
    wg = nc.dram_tensor("wg", [NI, P, NH * P], F32R, kind="ExternalInput")
    wi = nc.dram_tensor("wi", [NI, P, NH * P], F32R, kind="ExternalInput")
    wo = nc.dram_tensor("wo", [NI, P, H], F32R, kind="ExternalInput")
    sel = nc.dram_tensor("sel", [P, E], F32, kind="ExternalInput")
    out = nc.dram_tensor("out", [T, H], F32, kind="ExternalOutput")
    logits_out = nc.dram_tensor("logits", [T, E], F32, kind="ExternalOutput")

    from contextlib import ExitStack
    with ExitStack() as st:
            tc = st.enter_context(tile.TileContext(nc))
            pool = lambda name, bufs, **kw: st.enter_context(
                tc.tile_pool(name=name, bufs=bufs, **kw))
            consts = pool("consts", 1)
            flatp = pool("flatp", 9)
            wgp = pool("wgp", 3)
            wip = pool("wip", 3)
            wop = pool("wop", 10)
            hidp = pool("hidp", 9)
            evp = pool("evp", 4)
            accp = pool("accp", 17)
            outp = pool("outp", 4)
            rtp = pool("rtp", 4)
            flop = pool("flop", 3)
            ltp = pool("ltp", 3)
            psg = pool("psg", 2, space="PSUM")
            psv = pool("psv", 2, space="PSUM")
            pso = pool("pso", 2, space="PSUM")
            psT = pool("psT", 1, space="PSUM")
            psr = pool("psr", 1, space="PSUM")

            # ---- constants ----
            rkh_sb = consts.tile([P, NH, E], F32R)
            nc.sync.dma_start(out=rkh_sb[:], in_=rk_h[:].rearrange("h p e -> p h e"))
            rkl_sb = consts.tile([P, NH, E], F32R)
            nc.sync.dma_start(out=rkl_sb[:], in_=rk_l[:].rearrange("h p e -> p h e"))
            ident = consts.tile([P, P], F32)
            masks.make_identity(nc, ident[:])
            sel_sb = consts.tile([P, E], F32)
            nc.sync.dma_start(out=sel_sb[:], in_=sel[:])
            # per-token routing scalar for this core's expert, one column per
            # global t-tile
            rsc = consts.tile([P, NB * NT], F32)

            for blk in range(NB):
                t0 = blk * TB

                # ---- load this block's activation tiles [P(h), TB] ----
                ft = []
                for hh in range(NH):
                    f_t = flatp.tile([P, TB], F32R, name="ft")
                    nc.sync.dma_start(out=f_t[:], in_=flatT[hh, :, t0:t0 + TB])
                    ft.append(f_t)

                # ---- router logits, [E, t] layout, hi/lo-compensated ----
                # logits = hi@rk_h + hi@rk_l + lo@rk_h  (error ~2^-28)
                lts = []
                for tcx in range(TB // NC2):
                    sl = slice(tcx * NC2, (tcx + 1) * NC2)
                    ps_lt = psT.tile([E, NC2], F32, name="ps_lt")
                    for hh in range(NH):
                        fl = flop.tile([P, NC2], F32R, name="fl")
                        nc.sync.dma_start(
                            out=fl[:],
                            in_=flatT_lo[hh, :, t0 + tcx * NC2:
                                         t0 + (tcx + 1) * NC2])
                        nc.tensor.matmul(
                            ps_lt[:], lhsT=rkh_sb[:, hh, :], rhs=ft[hh][:, sl],
                            start=(hh == 0), stop=False)
                        nc.tensor.matmul(
                            ps_lt[:], lhsT=rkl_sb[:, hh, :], rhs=ft[hh][:, sl],
                            start=False, stop=False)
                        nc.tensor.matmul(
                            ps_lt[:], lhsT=rkh_sb[:, hh, :], rhs=fl[:],
                            start=False, stop=(hh == NH - 1))
                    lt_sb = ltp.tile([E, NC2], F32, name="lt_sb")
                    nc.vector.tensor_copy(lt_sb[:], ps_lt[:])
                    lts.append(lt_sb)

                for tt in range(NT):
                    g_tt = blk * NT + tt
                    tcx, off = divmod(tt * P, NC2)
                    ps_r = psr.tile([P, E], F32, name="ps_r")
                    nc.tensor.transpose(
                        ps_r[:], lts[tcx][:, off:off + P], ident[:E, :E])
                    L = rtp.tile([P, E], F32, name="L")
                    nc.vector.tensor_copy(L[:], ps_r[:])
                    nc.sync.dma_start(
                        out=logits_out[t0 + tt * P: t0 + (tt + 1) * P, :],
                        in_=L[:],
                    )
                    # top-2 + renormalize:  w1 = sigmoid(l1-l2), w2 = 1-w1
                    m1 = rtp.tile([P, 1], F32, name="m1")
                    nc.vector.tensor_reduce(m1[:], L[:], axis=AX.X, op=OP.max)
                    is1 = rtp.tile([P, E], F32, name="is1")
                    nc.vector.tensor_scalar(
                        out=is1[:], in0=L[:], scalar1=m1[:], scalar2=None,
                        op0=OP.is_ge,
                    )
                    big = rtp.tile([P, E], F32, name="big")
                    nc.vector.tensor_scalar_mul(big[:], is1[:], -1e30)
                    Lm = rtp.tile([P, E], F32, name="Lm")
                    nc.vector.tensor_add(Lm[:], L[:], big[:])
                    m2 = rtp.tile([P, 1], F32, name="m2")
                    nc.vector.tensor_reduce(m2[:], Lm[:], axis=AX.X, op=OP.max)
                    d12 = rtp.tile([P, 1], F32, name="d12")
                    nc.vector.tensor_sub(d12[:], m1[:], m2[:])
                    s1 = rtp.tile([P, 1], F32, name="s1")
                    nc.scalar.activation(s1[:], d12[:], AF.Sigmoid)
                    s2 = rtp.tile([P, 1], F32, name="s2")
                    nc.vector.tensor_scalar(
                        out=s2[:], in0=s1[:], scalar1=-1.0, scalar2=1.0,
                        op0=OP.mult, op1=OP.add,
                    )
                    sel2 = rtp.tile([P, E], F32, name="sel2")
                    nc.vector.tensor_scalar(
                        out=sel2[:], in0=L[:], scalar1=m2[:], scalar2=None,
                        op0=OP.is_ge,
                    )
                    is2 = rtp.tile([P, E], F32, name="is2")
                    nc.vector.tensor_sub(is2[:], sel2[:], is1[:])
                    w1 = rtp.tile([P, E], F32, name="w1")
                    nc.vector.tensor_scalar_mul(w1[:], is1[:], s1[:])
                    wr = rtp.tile([P, E], F32, name="wr")
                    nc.vector.scalar_tensor_tensor(
                        out=wr[:], in0=is2[:], scalar=s2[:], in1=w1[:],
                        op0=OP.mult, op1=OP.add,
                    )
                    wsel = rtp.tile([P, E], F32, name="wsel")
                    nc.vector.tensor_mul(wsel[:], wr[:], sel_sb[:])
                    nc.vector.tensor_reduce(
                        rsc[:, g_tt:g_tt + 1], wsel[:], axis=AX.X, op=OP.add,
                    )

                # ---- two half-sweeps over I: SwiGLU then down-proj partial ----
                acc = {}
                for half in range(2):
                    i_lo, i_hi = half * (NI // 2), (half + 1) * (NI // 2)

                    # phase 1: hiddenT tiles [P(i), TB] for this half
                    hid = {}
                    for ii in range(i_lo, i_hi):
                        wg_t = wgp.tile([P, NH * P], F32R, name="wg_t")
                        nc.sync.dma_start(out=wg_t[:], in_=wg[ii, :, :])
                        wi_t = wip.tile([P, NH * P], F32R, name="wi_t")
                        nc.sync.dma_start(out=wi_t[:], in_=wi[ii, :, :])
                        hid_t = hidp.tile([P, TB], F32R, name="hid_t")
                        for tcx in range(TB // NC2):
                            sl = slice(tcx * NC2, (tcx + 1) * NC2)
                            ps_g = psg.tile([P, NC2], F32, name="ps_g")
                            ps_v = psv.tile([P, NC2], F32, name="ps_v")
                            for hh in range(NH):
                                nc.tensor.matmul(
                                    ps_g[:],
                                    lhsT=wg_t[:, hh * P:(hh + 1) * P],
                                    rhs=ft[hh][:, sl],
                                    start=(hh == 0), stop=(hh == NH - 1),
                                )
                            for hh in range(NH):
                                nc.tensor.matmul(
                                    ps_v[:],
                                    lhsT=wi_t[:, hh * P:(hh + 1) * P],
                                    rhs=ft[hh][:, sl],
                                    start=(hh == 0), stop=(hh == NH - 1),
                                )
                            si_t = evp.tile([P, NC2], F32, name="si_t")
                            nc.scalar.activation(si_t[:], ps_g[:], AF.Silu)
                            nc.vector.tensor_mul(hid_t[:, sl], si_t[:], ps_v[:])
                        hid[ii] = hid_t

                    # phase 2: out[t,h] partial over this I-half
                    for ii in range(i_lo, i_hi):
                        wo_t = wop.tile([P, H], F32R, name="wo_t")
                        nc.sync.dma_start(out=wo_t[:], in_=wo[ii, :, :])
                        hid[ii] = (hid[ii], wo_t)

                    for tt in range(NT):
                        g_tt = blk * NT + tt
                        tsl = slice(tt * P, (tt + 1) * P)
                        for hcx in range(H // NC2):
                            hsl = slice(hcx * NC2, (hcx + 1) * NC2)
                            ps_o = pso.tile([P, NC2], F32, name="ps_o")
                            for ii in range(i_lo, i_hi):
                                hid_t, wo_t = hid[ii]
                                nc.tensor.matmul(
                                    ps_o[:],
                                    lhsT=hid_t[:, tsl],
                                    rhs=wo_t[:, hsl],
                                    start=(ii == i_lo), stop=(ii == i_hi - 1),
                                )
                            if half == 0:
                                # stash routing-scaled partial in SBUF
                                a_t = accp.tile([P, NC2], F32, name="a_t")
                                nc.vector.tensor_scalar_mul(
                                    a_t[:], ps_o[:], rsc[:, g_tt:g_tt + 1],
                                )
                                acc[(tt, hcx)] = a_t
                            else:
                                o_t = outp.tile([P, NC2], F32, name="o_t")
                                nc.vector.scalar_tensor_tensor(
                                    out=o_t[:], in0=ps_o[:],
                                    scalar=rsc[:, g_tt:g_tt + 1],
                                    in1=acc[(tt, hcx)][:],
                                    op0=OP.mult, op1=OP.add,
                                )
                                nc.sync.dma_start(
                                    out=out[t0 + tt * P: t0 + (tt + 1) * P, hsl],
                                    in_=o_t[:],
                                )
    nc.finalize()
    return nc


def _fp22_split(x):
    """x -> (hi, lo): hi = x truncated to 13 mantissa bits (exact FP22),
    lo = x - hi (exact in fp32)."""
    hi = (x.view(np.uint32) & np.uint32(0xFFFFE000)).view(np.float32)
    return hi, x - hi


def _prep_in_maps(hidden_states, router_kernel, w_gate, w_inter, w_out):
    f32 = np.float32
    flat = np.ascontiguousarray(hidden_states, dtype=f32).reshape(T, H)
    flatT_full = np.ascontiguousarray(flat.T)
    flatT, flatT_lo = _fp22_split(flatT_full)
    flatT = flatT.reshape(NH, P, T)
    flatT_lo = np.ascontiguousarray(flatT_lo).reshape(NH, P, T)
    rk = np.ascontiguousarray(router_kernel, dtype=f32)
    rk_h, rk_l = _fp22_split(rk)
    rk_h = rk_h.reshape(NH, P, E)
    rk_l = np.ascontiguousarray(rk_l).reshape(NH, P, E)
    in_maps = []
    for c in range(NCORES):
        wg_c = np.ascontiguousarray(
            np.asarray(w_gate[c], dtype=f32).reshape(NH, P, NI, P)
            .transpose(2, 1, 0, 3).reshape(NI, P, NH * P))
        wi_c = np.ascontiguousarray(
            np.asarray(w_inter[c], dtype=f32).reshape(NH, P, NI, P)
            .transpose(2, 1, 0, 3).reshape(NI, P, NH * P))
        wo_c = np.ascontiguousarray(
            np.asarray(w_out[c], dtype=f32).reshape(NI, P, H))
        sel_c = np.zeros((P, E), dtype=f32)
        sel_c[:, c] = 1.0
        in_maps.append({
            "flatT": flatT, "flatT_lo": flatT_lo, "rk_h": rk_h, "rk_l": rk_l,
            "wg": wg_c, "wi": wi_c, "wo": wo_c, "sel": sel_c,
        })
    return in_maps


def run_on_device(hidden_states, router_kernel, w_gate, w_inter, w_out,
                  trace=False, **trace_kw):
    """Build (cached), run SPMD on 8 cores; returns (out, logits, results)."""
    from concourse.bass_utils import run_bass_kernel_spmd

    if "nc" not in _CACHE:
        _CACHE["nc"] = _build_nc()
    nc = _CACHE["nc"]

    in_maps = _prep_in_maps(
        hidden_states, router_kernel, w_gate, w_inter, w_out)
    res = run_bass_kernel_spmd(
        nc, in_maps, core_ids=list(range(NCORES)), trace=trace, **trace_kw)
    parts = res.results
    out = parts[0]["out"].astype(np.float32, copy=True)
    for c in range(1, NCORES):
        out += parts[c]["out"]
    logits = parts[0]["logits"]
    return out.reshape(2, 2048, H), logits, res


def kernel(hidden_states, router_kernel, w_gate, w_inter, w_out):
    out, logits, _ = run_on_device(
        hidden_states, router_kernel, w_gate, w_inter, w_out)
    return out, logits


# revision 18
# speedup vs baseline: 26531.5239x; 2.1002x over previous
"""Trainium2 Bass kernel: Mixtral-style MoE block (8 experts, top-2 router).

Sharding: expert-parallel across 8 NeuronCores — core c owns expert c's
w_gate/w_inter/w_out. The router is replicated (every core computes logits +
top-2 weights and scales its own expert's output by its routing weight);
the weighted expert sum is realized as a host-side sum over the 8 per-core
partial outputs at gather time.

Per-core compute layout (all matmuls via PE in float32r = full rate):
  flatT [H, T] (host-pretransposed)  ->  gateT/interT tiles [I_tile, T]
  hiddenT = silu(gateT) * interT     ->  out [T, H] = hiddenT.T @ w_out
The second matmul uses hiddenT tiles directly as the stationary operand, so
no on-device transposes are needed anywhere.
"""

import numpy as np

# ---- problem constants (hardcoded per contract) ----
P = 128
T = 4096           # tokens (B*S = 2*2048)
H = 1024           # hidden
ID = 2048          # intermediate
E = 8              # experts
NH = H // P        # 8 h-tiles
NI = ID // P       # 16 i-tiles
NB = 4             # token blocks
TB = T // NB       # 1024 tokens per block
NT = TB // P       # 8 t-tiles per block
NC2 = 512          # matmul moving free-dim chunk
NCORES = 8

_CACHE = {}


def _build_nc():
    import concourse.bass as bass
    import concourse.bacc as bacc
    from concourse import mybir
    from concourse import tile
    from concourse import masks

    F32 = mybir.dt.float32
    F32R = mybir.dt.float32r
    AF = mybir.ActivationFunctionType
    OP = mybir.AluOpType
    AX = mybir.AxisListType

    nc = bacc.Bacc(None, target_bir_lowering=False)

    # DRAM I/O (host-pretiled layouts; see kernel() below)
    # flatT carries the FP22-truncated "hi" part; flatT_lo the residual, so
    # the router can form an exactly-compensated fp32 logit product.
    flatT = nc.dram_tensor("flatT", [NH, P, T], F32R, kind="ExternalInput")
    flatT_lo = nc.dram_tensor("flatT_lo", [NH, P, T], F32R, kind="ExternalInput")
    rk_h = nc.dram_tensor("rk_h", [NH, P, E], F32R, kind="ExternalInput")
    rk_l = nc.dram_tensor("rk_l", [NH, P, E], F32R, kind="ExternalInput")
    wg = nc.dram_tensor("wg", [NI, P, NH * P], F32R, kind="ExternalInput")
    wi = nc.dram_tensor("wi", [NI, P, NH * P], F32R, kind="ExternalInput")
    wo = nc.dram_tensor("wo", [NI, P, H], F32R, kind="ExternalInput")
    sel = nc.dram_tensor("sel", [P, E], F32, kind="ExternalInput")
    out = nc.dram_tensor("out", [T, H], F32, kind="ExternalOutput")
    logits_out = nc.dram_tensor("logits", [T, E], F32, kind="ExternalOutput")

    from contextlib import ExitStack
    with ExitStack() as st:
            tc = st.enter_context(tile.TileContext(nc))
            pool = lambda name, bufs, **kw: st.enter_context(
                tc.tile_pool(name=name, bufs=bufs, **kw))
            consts = pool("consts", 1)
            flatp = pool("flatp", 9)
            wgp = pool("wgp", 3)
            wip = pool("wip", 3)
            wop = pool("wop", 10)
            hidp = pool("hidp", 9)
            evp = pool("evp", 4)
            accp = pool("accp", 17)
            outp = pool("outp", 4)
            rtp = pool("rtp", 4)
            flop = pool("flop", 3)
            ltp = pool("ltp", 3)
            psg = pool("psg", 2, space="PSUM")
            psv = pool("psv", 2, space="PSUM")
            pso = pool("pso", 2, space="PSUM")
            psT = pool("psT", 1, space="PSUM")
            psr = pool("psr", 1, space="PSUM")

            # ---- constants ----
            rkh_sb = consts.tile([P, NH, E], F32R)
            nc.sync.dma_start(out=rkh_sb[:], in_=rk_h[:].rearrange("h p e -> p h e"))
            rkl_sb = consts.tile([P, NH, E], F32R)
            nc.sync.dma_start(out=rkl_sb[:], in_=rk_l[:].rearrange("h p e -> p h e"))
            ident = consts.tile([P, P], F32)
            masks.make_identity(nc, ident[:])
            sel_sb = consts.tile([P, E], F32)
            nc.sync.dma_start(out=sel_sb[:], in_=sel[:])
            # per-token routing scalar for this core's expert, one column per
            # global t-tile
            rsc = consts.tile([P, NB * NT], F32)

            for blk in range(NB):
                t0 = blk * TB

                # ---- load this block's activation tiles [P(h), TB] ----
                ft = []
                for hh in range(NH):
                    f_t = flatp.tile([P, TB], F32R, name="ft")
                    nc.sync.dma_start(out=f_t[:], in_=flatT[hh, :, t0:t0 + TB])
                    ft.append(f_t)

                # ---- router logits, [E, t] layout, hi/lo-compensated ----
                # logits = hi@rk_h + hi@rk_l + lo@rk_h  (error ~2^-28)
                lts = []
                for tcx in range(TB // NC2):
                    sl = slice(tcx * NC2, (tcx + 1) * NC2)
                    ps_lt = psT.tile([E, NC2], F32, name="ps_lt")
                    for hh in range(NH):
                        fl = flop.tile([P, NC2], F32R, name="fl")
                        nc.sync.dma_start(
                            out=fl[:],
                            in_=flatT_lo[hh, :, t0 + tcx * NC2:
                                         t0 + (tcx + 1) * NC2])
                        nc.tensor.matmul(
                            ps_lt[:], lhsT=rkh_sb[:, hh, :], rhs=ft[hh][:, sl],
                            start=(hh == 0), stop=False)
                        nc.tensor.matmul(
                            ps_lt[:], lhsT=rkl_sb[:, hh, :], rhs=ft[hh][:, sl],
                            start=False, stop=False)
                        nc.tensor.matmul(
                            ps_lt[:], lhsT=rkh_sb[:, hh, :], rhs=fl[:],
                            start=False, stop=(hh == NH - 1))
                    lt_sb = ltp.tile([E, NC2], F32, name="lt_sb")
                    nc.vector.tensor_copy(lt_sb[:], ps_lt[:])
                    lts.append(lt_sb)

                for tt in range(NT):
                    g_tt = blk * NT + tt
                    tcx, off = divmod(tt * P, NC2)
                    ps_r = psr.tile([P, E], F32, name="ps_r")
                    nc.tensor.transpose(
                        ps_r[:], lts[tcx][:, off:off + P], ident[:E, :E])
                    L = rtp.tile([P, E], F32, name="L")
                    nc.vector.tensor_copy(L[:], ps_r[:])
                    nc.sync.dma_start(
                        out=logits_out[t0 + tt * P: t0 + (tt + 1) * P, :],
                        in_=L[:],
                    )
                    # top-2 + renormalize:  w1 = sigmoid(l1-l2), w2 = 1-w1
                    m1 = rtp.tile([P, 1], F32, name="m1")
                    nc.vector.tensor_reduce(m1[:], L[:], axis=AX.X, op=OP.max)
                    is1 = rtp.tile([P, E], F32, name="is1")
                    nc.vector.tensor_scalar(
                        out=is1[:], in0=L[:], scalar1=m1[:], scalar2=None,
                        op0=OP.is_ge,
                    )
                    big = rtp.tile([P, E], F32, name="big")
                    nc.vector.tensor_scalar_mul(big[:], is1[:], -1e30)
                    Lm = rtp.tile([P, E], F32, name="Lm")
                    nc.vector.tensor_add(Lm[:], L[:], big[:])
                    m2 = rtp.tile([P, 1], F32, name="m2")
                    nc.vector.tensor_reduce(m2[:], Lm[:], axis=AX.X, op=OP.max)
                    d12 = rtp.tile([P, 1], F32, name="d12")
                    nc.vector.tensor_sub(d12[:], m1[:], m2[:])
                    s1 = rtp.tile([P, 1], F32, name="s1")
                    nc.scalar.activation(s1[:], d12[:], AF.Sigmoid)
                    s2 = rtp.tile([P, 1], F32, name="s2")
                    nc.vector.tensor_scalar(
                        out=s2[:], in0=s1[:], scalar1=-1.0, scalar2=1.0,
                        op0=OP.mult, op1=OP.add,
                    )
                    sel2 = rtp.tile([P, E], F32, name="sel2")
                    nc.vector.tensor_scalar(
                        out=sel2[:], in0=L[:], scalar1=m2[:], scalar2=None,
                        op0=OP.is_ge,
                    )
                    is2 = rtp.tile([P, E], F32, name="is2")
                    nc.vector.tensor_sub(is2[:], sel2[:], is1[:])
                    w1 = rtp.tile([P, E], F32, name="w1")
                    nc.vector.tensor_scalar_mul(w1[:], is1[:], s1[:])
                    wr = rtp.tile([P, E], F32, name="wr")
                    nc.vector.scalar_tensor_tensor(
                        out=wr[:], in0=is2[:], scalar=s2[:], in1=w1[:],
                        op0=OP.mult, op1=OP.add,
                    )
                    wsel = rtp.tile([P, E], F32, name="wsel")
                    nc.vector.tensor_mul(wsel[:], wr[:], sel_sb[:])
                    nc.vector.tensor_reduce(
                        rsc[:, g_tt:g_tt + 1], wsel[:], axis=AX.X, op=OP.add,
                    )

                # ---- two half-sweeps over I: SwiGLU then down-proj partial ----
                acc = {}
                for half in range(2):
                    i_lo, i_hi = half * (NI // 2), (half + 1) * (NI // 2)

                    # phase 1: hiddenT tiles [P(i), TB] for this half
                    hid = {}
                    for ii in range(i_lo, i_hi):
                        wg_t = wgp.tile([P, NH * P], F32R, name="wg_t")
                        nc.sync.dma_start(out=wg_t[:], in_=wg[ii, :, :])
                        wi_t = wip.tile([P, NH * P], F32R, name="wi_t")
                        nc.sync.dma_start(out=wi_t[:], in_=wi[ii, :, :])
                        hid_t = hidp.tile([P, TB], F32R, name="hid_t")
                        for tcx in range(TB // NC2):
                            sl = slice(tcx * NC2, (tcx + 1) * NC2)
                            ps_g = psg.tile([P, NC2], F32, name="ps_g")
                            ps_v = psv.tile([P, NC2], F32, name="ps_v")
                            for hh in range(NH):
                                nc.tensor.matmul(
                                    ps_g[:],
                                    lhsT=wg_t[:, hh * P:(hh + 1) * P],
                                    rhs=ft[hh][:, sl],
                                    start=(hh == 0), stop=(hh == NH - 1),
                                )
                            for hh in range(NH):
                                nc.tensor.matmul(
                                    ps_v[:],
                                    lhsT=wi_t[:, hh * P:(hh + 1) * P],
                                    rhs=ft[hh][:, sl],
                                    start=(hh == 0), stop=(hh == NH - 1),
                                )
                            si_t = evp.tile([P, NC2], F32, name="si_t")
                            nc.scalar.activation(si_t[:], ps_g[:], AF.Silu)
                            nc.vector.tensor_mul(hid_t[:, sl], si_t[:], ps_v[:])
                        hid[ii] = hid_t

                    # phase 2: out[t,h] partial over this I-half
                    for ii in range(i_lo, i_hi):
                        wo_t = wop.tile([P, H], F32R, name="wo_t")
                        nc.sync.dma_start(out=wo_t[:], in_=wo[ii, :, :])
                        hid[ii] = (hid[ii], wo_t)

                    for tt in range(NT):
                        g_tt = blk * NT + tt
                        tsl = slice(tt * P, (tt + 1) * P)
                        for hcx in range(H // NC2):
                            hsl = slice(hcx * NC2, (hcx + 1) * NC2)
                            ps_o = pso.tile([P, NC2], F32, name="ps_o")
                            for ii in range(i_lo, i_hi):
                                hid_t, wo_t = hid[ii]
                                nc.tensor.matmul(
                                    ps_o[:],
                                    lhsT=hid_t[:, tsl],
                                    rhs=wo_t[:, hsl],
                                    start=(ii == i_lo), stop=(ii == i_hi - 1),
                                )
                            if half == 0:
                                # stash routing-scaled partial in SBUF
                                a_t = accp.tile([P, NC2], F32, name="a_t")
                                nc.vector.tensor_scalar_mul(
                                    a_t[:], ps_o[:], rsc[:, g_tt:g_tt + 1],
                                )
                                acc[(tt, hcx)] = a_t
                            else:
                                o_t = outp.tile([P, NC2], F32, name="o_t")
                                nc.vector.scalar_tensor_tensor(
                                    out=o_t[:], in0=ps_o[:],
                                    scalar=rsc[:, g_tt:g_tt + 1],
                                    in1=acc[(tt, hcx)][:],
                                    op0=OP.mult, op1=OP.add,
                                )
                                nc.sync.dma_start(
                                    out=out[t0 + tt * P: t0 + (tt + 1) * P, hsl],
                                    in_=o_t[:],
                                )
    nc.finalize()
    return nc


def _build_nc_sparse(blocks):
    """Capacity-gathered variant: expert FFN computed only for this core's
    assigned tokens (host gathers per-expert token sets; zero-weight tokens
    contribute exactly 0 in the reference sum, so skipping them is exact).

    blocks: token-block sizes for the gathered dimension, e.g. [1024, 512]
    for capacity 1536. Full-T router logits are still computed on device
    ([E, T] layout, host transposes); per-gathered-token top-2 weights are
    computed on device from compensated logits.
    """
    import concourse.bass as bass
    import concourse.bacc as bacc
    from concourse import mybir
    from concourse import tile
    from concourse import masks
    from contextlib import ExitStack

    F32 = mybir.dt.float32
    F32R = mybir.dt.float32r
    AF = mybir.ActivationFunctionType
    OP = mybir.AluOpType
    AX = mybir.AxisListType

    CAP = sum(blocks)
    nc = bacc.Bacc(None, target_bir_lowering=False)

    flatT = nc.dram_tensor("flatT", [NH, P, T], F32R, kind="ExternalInput")
    fg = nc.dram_tensor("fg", [NH, P, CAP], F32R, kind="ExternalInput")
    fg_lo = nc.dram_tensor("fg_lo", [NH, P, CAP], F32R, kind="ExternalInput")
    rk_h = nc.dram_tensor("rk_h", [NH, P, E], F32R, kind="ExternalInput")
    rk_l = nc.dram_tensor("rk_l", [NH, P, E], F32R, kind="ExternalInput")
    wg = nc.dram_tensor("wg", [NI, P, NH * P], F32R, kind="ExternalInput")
    wi = nc.dram_tensor("wi", [NI, P, NH * P], F32R, kind="ExternalInput")
    wo = nc.dram_tensor("wo", [NI, P, H], F32R, kind="ExternalInput")
    sel = nc.dram_tensor("sel", [P, E], F32, kind="ExternalInput")
    out = nc.dram_tensor("out", [CAP, H], F32, kind="ExternalOutput")
    logitsT_out = nc.dram_tensor("logitsT", [E, T], F32, kind="ExternalOutput")

    with ExitStack() as st:
        tc = st.enter_context(tile.TileContext(nc))
        pool = lambda name, bufs, **kw: st.enter_context(
            tc.tile_pool(name=name, bufs=bufs, **kw))
        consts = pool("consts", 1)
        flatp = pool("flatp", 9)
        wgp = pool("wgp", 3)
        wip = pool("wip", 3)
        wop = pool("wop", 10)
        hidp = pool("hidp", 9)
        evp = pool("evp", 3)
        accp = pool("accp", 17)
        outp = pool("outp", 4)
        rtp = pool("rtp", 4)
        flop = pool("flop", 3)
        ltp = pool("ltp", 3)
        rfp = pool("rfp", 3)
        psg = pool("psg", 2, space="PSUM")
        psv = pool("psv", 2, space="PSUM")
        pso = pool("pso", 2, space="PSUM")
        psT = pool("psT", 1, space="PSUM")
        psr = pool("psr", 1, space="PSUM")

        rkh_sb = consts.tile([P, NH, E], F32R)
        nc.sync.dma_start(out=rkh_sb[:], in_=rk_h[:].rearrange("h p e -> p h e"))
        rkl_sb = consts.tile([P, NH, E], F32R)
        nc.sync.dma_start(out=rkl_sb[:], in_=rk_l[:].rearrange("h p e -> p h e"))
        ident = consts.tile([P, P], F32)
        masks.make_identity(nc, ident[:])
        sel_sb = consts.tile([P, E], F32)
        nc.sync.dma_start(out=sel_sb[:], in_=sel[:])
        rsc = consts.tile([P, 32], F32)

        # prefetch the first expert weight tiles so phase 1 of block 0 can
        # start right after the router instead of waiting behind the
        # activation loads in the DMA queue
        preloaded = {}
        for ii in range(2):
            pwg = wgp.tile([P, NH * P], F32R, name="wg_t")
            nc.sync.dma_start(out=pwg[:], in_=wg[ii, :, :])
            pwi = wip.tile([P, NH * P], F32R, name="wi_t")
            nc.sync.dma_start(out=pwi[:], in_=wi[ii, :, :])
            preloaded[ii] = (pwg, pwi)

        # full-T router logits chunks, interleaved into the expert schedule
        rf_remaining = list(range(T // NC2))

        def emit_rf_chunk():
            if not rf_remaining:
                return
            tcx = rf_remaining.pop(0)
            ps_lt = psT.tile([E, NC2], F32, name="ps_lt")
            for hh in range(NH):
                rf = rfp.tile([P, NC2], F32R, name="rf")
                nc.sync.dma_start(
                    out=rf[:],
                    in_=flatT[hh, :, tcx * NC2:(tcx + 1) * NC2])
                nc.tensor.matmul(
                    ps_lt[:], lhsT=rkh_sb[:, hh, :], rhs=rf[:],
                    start=(hh == 0), stop=(hh == NH - 1))
            lt_sb = ltp.tile([E, NC2], F32, name="lt_sb")
            nc.vector.tensor_copy(lt_sb[:], ps_lt[:])
            nc.sync.dma_start(
                out=logitsT_out[:, tcx * NC2:(tcx + 1) * NC2], in_=lt_sb[:])

        t0g = 0
        for blk, TBg in enumerate(blocks):
            NTg = TBg // P

            ft = []
            for hh in range(NH):
                f_t = flatp.tile([P, TB], F32R, name="ft")
                nc.sync.dma_start(
                    out=f_t[:, :TBg], in_=fg[hh, :, t0g:t0g + TBg])
                ft.append(f_t)

            # gathered-token router: compensated logits -> top-2 weights
            lts = []
            for tcx in range(TBg // NC2):
                sl = slice(tcx * NC2, (tcx + 1) * NC2)
                ps_lt = psT.tile([E, NC2], F32, name="ps_lt")
                for hh in range(NH):
                    fl = flop.tile([P, NC2], F32R, name="fl")
                    nc.sync.dma_start(
                        out=fl[:],
                        in_=fg_lo[hh, :, t0g + tcx * NC2:
                                  t0g + (tcx + 1) * NC2])
                    nc.tensor.matmul(
                        ps_lt[:], lhsT=rkh_sb[:, hh, :], rhs=ft[hh][:, sl],
                        start=(hh == 0), stop=False)
                    nc.tensor.matmul(
                        ps_lt[:], lhsT=rkl_sb[:, hh, :], rhs=ft[hh][:, sl],
                        start=False, stop=False)
                    nc.tensor.matmul(
                        ps_lt[:], lhsT=rkh_sb[:, hh, :], rhs=fl[:],
                        start=False, stop=(hh == NH - 1))
                lt_sb = ltp.tile([E, NC2], F32, name="lt_sb")
                nc.vector.tensor_copy(lt_sb[:], ps_lt[:])
                lts.append(lt_sb)

            for tt in range(NTg):
                g_tt = (t0g + tt * P) // P
                tcx, off = divmod(tt * P, NC2)
                ps_r = psr.tile([P, E], F32, name="ps_r")
                nc.tensor.transpose(
                    ps_r[:], lts[tcx][:, off:off + P], ident[:E, :E])
                L = rtp.tile([P, E], F32, name="L")
                nc.vector.tensor_copy(L[:], ps_r[:])
                m1 = rtp.tile([P, 1], F32, name="m1")
                nc.vector.tensor_reduce(m1[:], L[:], axis=AX.X, op=OP.max)
                is1 = rtp.tile([P, E], F32, name="is1")
                nc.vector.tensor_scalar(
                    out=is1[:], in0=L[:], scalar1=m1[:], scalar2=None,
                    op0=OP.is_ge)
                big = rtp.tile([P, E], F32, name="big")
                nc.vector.tensor_scalar_mul(big[:], is1[:], -1e30)
                Lm = rtp.tile([P, E], F32, name="Lm")
                nc.vector.tensor_add(Lm[:], L[:], big[:])
                m2 = rtp.tile([P, 1], F32, name="m2")
                nc.vector.tensor_reduce(m2[:], Lm[:], axis=AX.X, op=OP.max)
                d12 = rtp.tile([P, 1], F32, name="d12")
                nc.vector.tensor_sub(d12[:], m1[:], m2[:])
                s1 = rtp.tile([P, 1], F32, name="s1")
                nc.scalar.activation(s1[:], d12[:], AF.Sigmoid)
                s2 = rtp.tile([P, 1], F32, name="s2")
                nc.vector.tensor_scalar(
                    out=s2[:], in0=s1[:], scalar1=-1.0, scalar2=1.0,
                    op0=OP.mult, op1=OP.add)
                sel2 = rtp.tile([P, E], F32, name="sel2")
                nc.vector.tensor_scalar(
                    out=sel2[:], in0=L[:], scalar1=m2[:], scalar2=None,
                    op0=OP.is_ge)
                is2 = rtp.tile([P, E], F32, name="is2")
                nc.vector.tensor_sub(is2[:], sel2[:], is1[:])
                w1 = rtp.tile([P, E], F32, name="w1")
                nc.vector.tensor_scalar_mul(w1[:], is1[:], s1[:])
                wr = rtp.tile([P, E], F32, name="wr")
                nc.vector.scalar_tensor_tensor(
                    out=wr[:], in0=is2[:], scalar=s2[:], in1=w1[:],
                    op0=OP.mult, op1=OP.add)
                wsel = rtp.tile([P, E], F32, name="wsel")
                nc.vector.tensor_mul(wsel[:], wr[:], sel_sb[:])
                nc.vector.tensor_reduce(
                    rsc[:, g_tt:g_tt + 1], wsel[:], axis=AX.X, op=OP.add)

            emit_rf_chunk()

            acc = {}
            for half in range(2):
                i_lo, i_hi = half * (NI // 2), (half + 1) * (NI // 2)

                hid = {}
                for ii in range(i_lo, i_hi):
                    if blk == 0 and half == 0 and ii in preloaded:
                        wg_t, wi_t = preloaded[ii]
                    else:
                        wg_t = wgp.tile([P, NH * P], F32R, name="wg_t")
                        nc.sync.dma_start(out=wg_t[:], in_=wg[ii, :, :])
                        wi_t = wip.tile([P, NH * P], F32R, name="wi_t")
                        nc.sync.dma_start(out=wi_t[:], in_=wi[ii, :, :])
                    hid_t = hidp.tile([P, TB], F32R, name="hid_t")
                    for tcx in range(TBg // NC2):
                        sl = slice(tcx * NC2, (tcx + 1) * NC2)
                        ps_g = psg.tile([P, NC2], F32, name="ps_g")
                        ps_v = psv.tile([P, NC2], F32, name="ps_v")
                        for hh in range(NH):
                            nc.tensor.matmul(
                                ps_g[:],
                                lhsT=wg_t[:, hh * P:(hh + 1) * P],
                                rhs=ft[hh][:, sl],
                                start=(hh == 0), stop=(hh == NH - 1))
                        for hh in range(NH):
                            nc.tensor.matmul(
                                ps_v[:],
                                lhsT=wi_t[:, hh * P:(hh + 1) * P],
                                rhs=ft[hh][:, sl],
                                start=(hh == 0), stop=(hh == NH - 1))
                        si_t = evp.tile([P, NC2], F32, name="si_t")
                        nc.scalar.activation(si_t[:], ps_g[:], AF.Silu)
                        nc.vector.tensor_mul(hid_t[:, sl], si_t[:], ps_v[:])
                    hid[ii] = hid_t

                for ii in range(i_lo, i_hi):
                    wo_t = wop.tile([P, H], F32R, name="wo_t")
                    nc.sync.dma_start(out=wo_t[:], in_=wo[ii, :, :])
                    hid[ii] = (hid[ii], wo_t)

                for tt in range(NTg):
                    g_tt = (t0g + tt * P) // P
                    tsl = slice(tt * P, (tt + 1) * P)
                    for hcx in range(H // NC2):
                        hsl = slice(hcx * NC2, (hcx + 1) * NC2)
                        ps_o = pso.tile([P, NC2], F32, name="ps_o")
                        for ii in range(i_lo, i_hi):
                            hid_t, wo_t = hid[ii]
                            nc.tensor.matmul(
                                ps_o[:],
                                lhsT=hid_t[:, tsl],
                                rhs=wo_t[:, hsl],
                                start=(ii == i_lo), stop=(ii == i_hi - 1))
                        if half == 0:
                            a_t = accp.tile([P, NC2], F32, name="a_t")
                            nc.vector.tensor_scalar_mul(
                                a_t[:], ps_o[:], rsc[:, g_tt:g_tt + 1])
                            acc[(tt, hcx)] = a_t
                        else:
                            o_t = outp.tile([P, NC2], F32, name="o_t")
                            nc.vector.scalar_tensor_tensor(
                                out=o_t[:], in0=ps_o[:],
                                scalar=rsc[:, g_tt:g_tt + 1],
                                in1=acc[(tt, hcx)][:],
                                op0=OP.mult, op1=OP.add)
                            nc.sync.dma_start(
                                out=out[t0g + tt * P: t0g + (tt + 1) * P,
                                        hsl],
                                in_=o_t[:])
                emit_rf_chunk()
            emit_rf_chunk()
            t0g += TBg
        while rf_remaining:
            emit_rf_chunk()
    nc.finalize()
    return nc


def _fp22_split(x):
    """x -> (hi, lo): hi = x truncated to 13 mantissa bits (exact FP22),
    lo = x - hi (exact in fp32)."""
    hi = (x.view(np.uint32) & np.uint32(0xFFFFE000)).view(np.float32)
    return hi, x - hi


def _prep_in_maps(hidden_states, router_kernel, w_gate, w_inter, w_out):
    f32 = np.float32
    flat = np.ascontiguousarray(hidden_states, dtype=f32).reshape(T, H)
    flatT_full = np.ascontiguousarray(flat.T)
    flatT, flatT_lo = _fp22_split(flatT_full)
    flatT = flatT.reshape(NH, P, T)
    flatT_lo = np.ascontiguousarray(flatT_lo).reshape(NH, P, T)
    rk = np.ascontiguousarray(router_kernel, dtype=f32)
    rk_h, rk_l = _fp22_split(rk)
    rk_h = rk_h.reshape(NH, P, E)
    rk_l = np.ascontiguousarray(rk_l).reshape(NH, P, E)
    in_maps = []
    for c in range(NCORES):
        wg_c = np.ascontiguousarray(
            np.asarray(w_gate[c], dtype=f32).reshape(NH, P, NI, P)
            .transpose(2, 1, 0, 3).reshape(NI, P, NH * P))
        wi_c = np.ascontiguousarray(
            np.asarray(w_inter[c], dtype=f32).reshape(NH, P, NI, P)
            .transpose(2, 1, 0, 3).reshape(NI, P, NH * P))
        wo_c = np.ascontiguousarray(
            np.asarray(w_out[c], dtype=f32).reshape(NI, P, H))
        sel_c = np.zeros((P, E), dtype=f32)
        sel_c[:, c] = 1.0
        in_maps.append({
            "flatT": flatT, "flatT_lo": flatT_lo, "rk_h": rk_h, "rk_l": rk_l,
            "wg": wg_c, "wi": wi_c, "wo": wo_c, "sel": sel_c,
        })
    return in_maps


def _expert_weight_maps(w_gate, w_inter, w_out):
    f32 = np.float32
    maps = []
    for c in range(NCORES):
        wg_c = np.ascontiguousarray(
            np.asarray(w_gate[c], dtype=f32).reshape(NH, P, NI, P)
            .transpose(2, 1, 0, 3).reshape(NI, P, NH * P))
        wi_c = np.ascontiguousarray(
            np.asarray(w_inter[c], dtype=f32).reshape(NH, P, NI, P)
            .transpose(2, 1, 0, 3).reshape(NI, P, NH * P))
        wo_c = np.ascontiguousarray(
            np.asarray(w_out[c], dtype=f32).reshape(NI, P, H))
        sel_c = np.zeros((P, E), dtype=f32)
        sel_c[:, c] = 1.0
        maps.append({"wg": wg_c, "wi": wi_c, "wo": wo_c, "sel": sel_c})
    return maps


def run_on_device(hidden_states, router_kernel, w_gate, w_inter, w_out,
                  trace=False, force_dense=False, **trace_kw):
    """Shard + run SPMD on 8 cores; returns (out, logits, results)."""
    from concourse.bass_utils import run_bass_kernel_spmd

    f32 = np.float32
    flat = np.ascontiguousarray(hidden_states, dtype=f32).reshape(T, H)
    rk = np.ascontiguousarray(router_kernel, dtype=f32)
    rk_h, rk_l = _fp22_split(rk)
    rk_h = rk_h.reshape(NH, P, E)
    rk_l = np.ascontiguousarray(rk_l).reshape(NH, P, E)

    # host-side dispatch (sharding decision): exact top-2 per token
    logits64 = flat.astype(np.float64) @ rk.astype(np.float64)
    srt = np.sort(logits64, -1)
    margin = (srt[:, -2] - srt[:, -3]).min()
    thr = srt[:, -2:-1]                      # 2nd-largest logit per token
    sel_mask = logits64 >= thr               # [T, E] top-2 membership
    counts = sel_mask.sum(0)
    cap_raw = int(counts.max())
    use_sparse = (not force_dense) and margin > 1e-5 and cap_raw <= 3584

    wmaps = _expert_weight_maps(w_gate, w_inter, w_out)

    if use_sparse:
        cap = max(512, ((cap_raw + 511) // 512) * 512)
        blocks = [1024] * (cap // 1024) + ([512] if cap % 1024 else [])
        key = ("sparse", cap)
        if key not in _CACHE:
            _CACHE[key] = _build_nc_sparse(blocks)
        nc = _CACHE[key]

        flatT_full = np.ascontiguousarray(flat.T).reshape(NH, P, T)
        idxs, in_maps = [], []
        for c in range(NCORES):
            idx = np.nonzero(sel_mask[:, c])[0]
            idxs.append(idx)
            gath = np.zeros((cap, H), f32)
            gath[:len(idx)] = flat[idx]
            gT = np.ascontiguousarray(gath.T)
            g_hi, g_lo = _fp22_split(gT)
            in_maps.append({
                "flatT": flatT_full,
                "fg": g_hi.reshape(NH, P, cap),
                "fg_lo": np.ascontiguousarray(g_lo).reshape(NH, P, cap),
                "rk_h": rk_h, "rk_l": rk_l, **wmaps[c],
            })
        res = run_bass_kernel_spmd(
            nc, in_maps, core_ids=list(range(NCORES)), trace=trace,
            **trace_kw)
        parts = res.results
        out = np.zeros((T, H), f32)
        for c in range(NCORES):
            n = len(idxs[c])
            out[idxs[c]] += parts[c]["out"][:n]
        logits = np.ascontiguousarray(parts[0]["logitsT"].T)
    else:
        key = ("dense",)
        if key not in _CACHE:
            _CACHE[key] = _build_nc()
        nc = _CACHE[key]
        flatT_full, flatT_lo = _fp22_split(np.ascontiguousarray(flat.T))
        in_maps = []
        for c in range(NCORES):
            in_maps.append({
                "flatT": flatT_full.reshape(NH, P, T),
                "flatT_lo": np.ascontiguousarray(flatT_lo).reshape(NH, P, T),
                "rk_h": rk_h, "rk_l": rk_l, **wmaps[c],
            })
        res = run_bass_kernel_spmd(
            nc, in_maps, core_ids=list(range(NCORES)), trace=trace,
            **trace_kw)
        parts = res.results
        out = parts[0]["out"].astype(np.float32, copy=True)
        for c in range(1, NCORES):
            out += parts[c]["out"]
        logits = parts[0]["logits"]
    return out.reshape(2, 2048, H), logits, res


def kernel(hidden_states, router_kernel, w_gate, w_inter, w_out):
    out, logits, _ = run_on_device(
        hidden_states, router_kernel, w_gate, w_inter, w_out)
    return out, logits


# revision 22
# speedup vs baseline: 28252.7869x; 1.0649x over previous
"""Trainium2 Bass kernel: Mixtral-style MoE block (8 experts, top-2 router).

Sharding: expert-parallel across 8 NeuronCores — core c owns expert c's
w_gate/w_inter/w_out. The router is replicated (every core computes logits +
top-2 weights and scales its own expert's output by its routing weight);
the weighted expert sum is realized as a host-side sum over the 8 per-core
partial outputs at gather time.

Per-core compute layout (all matmuls via PE in float32r = full rate):
  flatT [H, T] (host-pretransposed)  ->  gateT/interT tiles [I_tile, T]
  hiddenT = silu(gateT) * interT     ->  out [T, H] = hiddenT.T @ w_out
The second matmul uses hiddenT tiles directly as the stationary operand, so
no on-device transposes are needed anywhere.
"""

import numpy as np

# ---- problem constants (hardcoded per contract) ----
P = 128
T = 4096           # tokens (B*S = 2*2048)
H = 1024           # hidden
ID = 2048          # intermediate
E = 8              # experts
NH = H // P        # 8 h-tiles
NI = ID // P       # 16 i-tiles
NB = 4             # token blocks
TB = T // NB       # 1024 tokens per block
NT = TB // P       # 8 t-tiles per block
NC2 = 512          # matmul moving free-dim chunk
NCORES = 8

_CACHE = {}


def _build_nc():
    import concourse.bass as bass
    import concourse.bacc as bacc
    from concourse import mybir
    from concourse import tile
    from concourse import masks

    F32 = mybir.dt.float32
    F32R = mybir.dt.float32r
    AF = mybir.ActivationFunctionType
    OP = mybir.AluOpType
    AX = mybir.AxisListType

    nc = bacc.Bacc(None, target_bir_lowering=False)

    # DRAM I/O (host-pretiled layouts; see kernel() below)
    # flatT carries the FP22-truncated "hi" part; flatT_lo the residual, so
    # the router can form an exactly-compensated fp32 logit product.
    flatT = nc.dram_tensor("flatT", [NH, P, T], F32R, kind="ExternalInput")
    flatT_lo = nc.dram_tensor("flatT_lo", [NH, P, T], F32R, kind="ExternalInput")
    rk_h = nc.dram_tensor("rk_h", [NH, P, E], F32R, kind="ExternalInput")
    rk_l = nc.dram_tensor("rk_l", [NH, P, E], F32R, kind="ExternalInput")
    wg = nc.dram_tensor("wg", [NI, P, NH * P], F32R, kind="ExternalInput")
    wi = nc.dram_tensor("wi", [NI, P, NH * P], F32R, kind="ExternalInput")
    wo = nc.dram_tensor("wo", [NI, P, H], F32R, kind="ExternalInput")
    sel = nc.dram_tensor("sel", [P, E], F32, kind="ExternalInput")
    out = nc.dram_tensor("out", [T, H], F32, kind="ExternalOutput")
    logits_out = nc.dram_tensor("logits", [T, E], F32, kind="ExternalOutput")

    from contextlib import ExitStack
    with ExitStack() as st:
            tc = st.enter_context(tile.TileContext(nc))
            pool = lambda name, bufs, **kw: st.enter_context(
                tc.tile_pool(name=name, bufs=bufs, **kw))
            consts = pool("consts", 1)
            flatp = pool("flatp", 9)
            wgp = pool("wgp", 3)
            wip = pool("wip", 3)
            wop = pool("wop", 10)
            hidp = pool("hidp", 9)
            evp = pool("evp", 4)
            accp = pool("accp", 17)
            outp = pool("outp", 4)
            rtp = pool("rtp", 4)
            flop = pool("flop", 3)
            ltp = pool("ltp", 3)
            psg = pool("psg", 2, space="PSUM")
            psv = pool("psv", 2, space="PSUM")
            pso = pool("pso", 2, space="PSUM")
            psT = pool("psT", 1, space="PSUM")
            psr = pool("psr", 1, space="PSUM")

            # ---- constants ----
            rkh_sb = consts.tile([P, NH, E], F32R)
            nc.sync.dma_start(out=rkh_sb[:], in_=rk_h[:].rearrange("h p e -> p h e"))
            rkl_sb = consts.tile([P, NH, E], F32R)
            nc.sync.dma_start(out=rkl_sb[:], in_=rk_l[:].rearrange("h p e -> p h e"))
            ident = consts.tile([P, P], F32)
            masks.make_identity(nc, ident[:])
            sel_sb = consts.tile([P, E], F32)
            nc.sync.dma_start(out=sel_sb[:], in_=sel[:])
            # per-token routing scalar for this core's expert, one column per
            # global t-tile
            rsc = consts.tile([P, NB * NT], F32)

            for blk in range(NB):
                t0 = blk * TB

                # ---- load this block's activation tiles [P(h), TB] ----
                ft = []
                for hh in range(NH):
                    f_t = flatp.tile([P, TB], F32R, name="ft")
                    nc.sync.dma_start(out=f_t[:], in_=flatT[hh, :, t0:t0 + TB])
                    ft.append(f_t)

                # ---- router logits, [E, t] layout, hi/lo-compensated ----
                # logits = hi@rk_h + hi@rk_l + lo@rk_h  (error ~2^-28)
                lts = []
                for tcx in range(TB // NC2):
                    sl = slice(tcx * NC2, (tcx + 1) * NC2)
                    ps_lt = psT.tile([E, NC2], F32, name="ps_lt")
                    for hh in range(NH):
                        fl = flop.tile([P, NC2], F32R, name="fl")
                        nc.sync.dma_start(
                            out=fl[:],
                            in_=flatT_lo[hh, :, t0 + tcx * NC2:
                                         t0 + (tcx + 1) * NC2])
                        nc.tensor.matmul(
                            ps_lt[:], lhsT=rkh_sb[:, hh, :], rhs=ft[hh][:, sl],
                            start=(hh == 0), stop=False)
                        nc.tensor.matmul(
                            ps_lt[:], lhsT=rkl_sb[:, hh, :], rhs=ft[hh][:, sl],
                            start=False, stop=False)
                        nc.tensor.matmul(
                            ps_lt[:], lhsT=rkh_sb[:, hh, :], rhs=fl[:],
                            start=False, stop=(hh == NH - 1))
                    lt_sb = ltp.tile([E, NC2], F32, name="lt_sb")
                    nc.vector.tensor_copy(lt_sb[:], ps_lt[:])
                    lts.append(lt_sb)

                for tt in range(NT):
                    g_tt = blk * NT + tt
                    tcx, off = divmod(tt * P, NC2)
                    ps_r = psr.tile([P, E], F32, name="ps_r")
                    nc.tensor.transpose(
                        ps_r[:], lts[tcx][:, off:off + P], ident[:E, :E])
                    L = rtp.tile([P, E], F32, name="L")
                    nc.vector.tensor_copy(L[:], ps_r[:])
                    nc.sync.dma_start(
                        out=logits_out[t0 + tt * P: t0 + (tt + 1) * P, :],
                        in_=L[:],
                    )
                    # top-2 + renormalize:  w1 = sigmoid(l1-l2), w2 = 1-w1
                    m1 = rtp.tile([P, 1], F32, name="m1")
                    nc.vector.tensor_reduce(m1[:], L[:], axis=AX.X, op=OP.max)
                    is1 = rtp.tile([P, E], F32, name="is1")
                    nc.vector.tensor_scalar(
                        out=is1[:], in0=L[:], scalar1=m1[:], scalar2=None,
                        op0=OP.is_ge,
                    )
                    big = rtp.tile([P, E], F32, name="big")
                    nc.vector.tensor_scalar_mul(big[:], is1[:], -1e30)
                    Lm = rtp.tile([P, E], F32, name="Lm")
                    nc.vector.tensor_add(Lm[:], L[:], big[:])
                    m2 = rtp.tile([P, 1], F32, name="m2")
                    nc.vector.tensor_reduce(m2[:], Lm[:], axis=AX.X, op=OP.max)
                    d12 = rtp.tile([P, 1], F32, name="d12")
                    nc.vector.tensor_sub(d12[:], m1[:], m2[:])
                    s1 = rtp.tile([P, 1], F32, name="s1")
                    nc.scalar.activation(s1[:], d12[:], AF.Sigmoid)
                    s2 = rtp.tile([P, 1], F32, name="s2")
                    nc.vector.tensor_scalar(
                        out=s2[:], in0=s1[:], scalar1=-1.0, scalar2=1.0,
                        op0=OP.mult, op1=OP.add,
                    )
                    sel2 = rtp.tile([P, E], F32, name="sel2")
                    nc.vector.tensor_scalar(
                        out=sel2[:], in0=L[:], scalar1=m2[:], scalar2=None,
                        op0=OP.is_ge,
                    )
                    is2 = rtp.tile([P, E], F32, name="is2")
                    nc.vector.tensor_sub(is2[:], sel2[:], is1[:])
                    w1 = rtp.tile([P, E], F32, name="w1")
                    nc.vector.tensor_scalar_mul(w1[:], is1[:], s1[:])
                    wr = rtp.tile([P, E], F32, name="wr")
                    nc.vector.scalar_tensor_tensor(
                        out=wr[:], in0=is2[:], scalar=s2[:], in1=w1[:],
                        op0=OP.mult, op1=OP.add,
                    )
                    wsel = rtp.tile([P, E], F32, name="wsel")
                    nc.vector.tensor_mul(wsel[:], wr[:], sel_sb[:])
                    nc.vector.tensor_reduce(
                        rsc[:, g_tt:g_tt + 1], wsel[:], axis=AX.X, op=OP.add,
                    )

                # ---- two half-sweeps over I: SwiGLU then down-proj partial ----
                acc = {}
                for half in range(2):
                    i_lo, i_hi = half * (NI // 2), (half + 1) * (NI // 2)

                    # phase 1: hiddenT tiles [P(i), TB] for this half
                    hid = {}
                    for ii in range(i_lo, i_hi):
                        wg_t = wgp.tile([P, NH * P], F32R, name="wg_t")
                        nc.sync.dma_start(out=wg_t[:], in_=wg[ii, :, :])
                        wi_t = wip.tile([P, NH * P], F32R, name="wi_t")
                        nc.sync.dma_start(out=wi_t[:], in_=wi[ii, :, :])
                        hid_t = hidp.tile([P, TB], F32R, name="hid_t")
                        for tcx in range(TB // NC2):
                            sl = slice(tcx * NC2, (tcx + 1) * NC2)
                            ps_g = psg.tile([P, NC2], F32, name="ps_g")
                            ps_v = psv.tile([P, NC2], F32, name="ps_v")
                            for hh in range(NH):
                                nc.tensor.matmul(
                                    ps_g[:],
                                    lhsT=wg_t[:, hh * P:(hh + 1) * P],
                                    rhs=ft[hh][:, sl],
                                    start=(hh == 0), stop=(hh == NH - 1),
                                )
                            for hh in range(NH):
                                nc.tensor.matmul(
                                    ps_v[:],
                                    lhsT=wi_t[:, hh * P:(hh + 1) * P],
                                    rhs=ft[hh][:, sl],
                                    start=(hh == 0), stop=(hh == NH - 1),
                                )
                            si_t = evp.tile([P, NC2], F32, name="si_t")
                            nc.scalar.activation(si_t[:], ps_g[:], AF.Silu)
                            nc.vector.tensor_mul(hid_t[:, sl], si_t[:], ps_v[:])
                        hid[ii] = hid_t

                    # phase 2: out[t,h] partial over this I-half
                    for ii in range(i_lo, i_hi):
                        wo_t = wop.tile([P, H], F32R, name="wo_t")
                        nc.sync.dma_start(out=wo_t[:], in_=wo[ii, :, :])
                        hid[ii] = (hid[ii], wo_t)

                    for tt in range(NT):
                        g_tt = blk * NT + tt
                        tsl = slice(tt * P, (tt + 1) * P)
                        for hcx in range(H // NC2):
                            hsl = slice(hcx * NC2, (hcx + 1) * NC2)
                            ps_o = pso.tile([P, NC2], F32, name="ps_o")
                            for ii in range(i_lo, i_hi):
                                hid_t, wo_t = hid[ii]
                                nc.tensor.matmul(
                                    ps_o[:],
                                    lhsT=hid_t[:, tsl],
                                    rhs=wo_t[:, hsl],
                                    start=(ii == i_lo), stop=(ii == i_hi - 1),
                                )
                            if half == 0:
                                # stash routing-scaled partial in SBUF
                                a_t = accp.tile([P, NC2], F32, name="a_t")
                                nc.vector.tensor_scalar_mul(
                                    a_t[:], ps_o[:], rsc[:, g_tt:g_tt + 1],
                                )
                                acc[(tt, hcx)] = a_t
                            else:
                                o_t = outp.tile([P, NC2], F32, name="o_t")
                                nc.vector.scalar_tensor_tensor(
                                    out=o_t[:], in0=ps_o[:],
                                    scalar=rsc[:, g_tt:g_tt + 1],
                                    in1=acc[(tt, hcx)][:],
                                    op0=OP.mult, op1=OP.add,
                                )
                                nc.sync.dma_start(
                                    out=out[t0 + tt * P: t0 + (tt + 1) * P, hsl],
                                    in_=o_t[:],
                                )
    nc.finalize()
    return nc


def _build_nc_sparse(blocks):
    """Capacity-gathered variant: expert FFN computed only for this core's
    assigned tokens (host gathers per-expert token sets; zero-weight tokens
    contribute exactly 0 in the reference sum, so skipping them is exact).

    blocks: token-block sizes for the gathered dimension, e.g. [1024, 512]
    for capacity 1536. Full-T router logits are still computed on device
    ([E, T] layout, host transposes); per-gathered-token top-2 weights are
    computed on device from compensated logits.
    """
    import concourse.bass as bass
    import concourse.bacc as bacc
    from concourse import mybir
    from concourse import tile
    from concourse import masks
    from contextlib import ExitStack

    F32 = mybir.dt.float32
    F32R = mybir.dt.float32r
    AF = mybir.ActivationFunctionType
    OP = mybir.AluOpType
    AX = mybir.AxisListType

    CAP = sum(blocks)
    nc = bacc.Bacc(None, target_bir_lowering=False)

    flatT = nc.dram_tensor("flatT", [NH, P, T], F32R, kind="ExternalInput")
    fg = nc.dram_tensor("fg", [NH, P, CAP], F32R, kind="ExternalInput")
    fg_lo = nc.dram_tensor("fg_lo", [NH, P, CAP], F32R, kind="ExternalInput")
    rk_h = nc.dram_tensor("rk_h", [NH, P, E], F32R, kind="ExternalInput")
    rk_l = nc.dram_tensor("rk_l", [NH, P, E], F32R, kind="ExternalInput")
    wg = nc.dram_tensor("wg", [NI, P, NH * P], F32R, kind="ExternalInput")
    wi = nc.dram_tensor("wi", [NI, P, NH * P], F32R, kind="ExternalInput")
    wo = nc.dram_tensor("wo", [NI, P, H], F32R, kind="ExternalInput")
    sel = nc.dram_tensor("sel", [P, E], F32, kind="ExternalInput")
    out = nc.dram_tensor("out", [CAP, H], F32, kind="ExternalOutput")
    logitsT_out = nc.dram_tensor("logitsT", [E, T], F32, kind="ExternalOutput")

    with ExitStack() as st:
        tc = st.enter_context(tile.TileContext(nc))
        pool = lambda name, bufs, **kw: st.enter_context(
            tc.tile_pool(name=name, bufs=bufs, **kw))
        consts = pool("consts", 1)
        flatp = pool("flatp", 9)
        wgp = pool("wgp", 3)
        wip = pool("wip", 3)
        wop = pool("wop", 10)
        hidp = pool("hidp", 9)
        evp = pool("evp", 3)
        accp = pool("accp", 17)
        outp = pool("outp", 4)
        rtp = pool("rtp", 4)
        flop = pool("flop", 3)
        ltp = pool("ltp", 3)
        rfp = pool("rfp", 3)
        psg = pool("psg", 2, space="PSUM")
        psv = pool("psv", 2, space="PSUM")
        pso = pool("pso", 2, space="PSUM")
        psT = pool("psT", 1, space="PSUM")
        psr = pool("psr", 1, space="PSUM")

        rkh_sb = consts.tile([P, NH, E], F32R)
        nc.sync.dma_start(out=rkh_sb[:], in_=rk_h[:].rearrange("h p e -> p h e"))
        rkl_sb = consts.tile([P, NH, E], F32R)
        nc.sync.dma_start(out=rkl_sb[:], in_=rk_l[:].rearrange("h p e -> p h e"))
        ident = consts.tile([P, P], F32)
        masks.make_identity(nc, ident[:])
        sel_sb = consts.tile([P, E], F32)
        nc.sync.dma_start(out=sel_sb[:], in_=sel[:])
        rsc = consts.tile([P, 32], F32)

        # prefetch the first expert weight tiles so phase 1 of block 0 can
        # start right after the router instead of waiting behind the
        # activation loads in the DMA queue
        preloaded = {}
        for ii in range(2):
            pwg = wgp.tile([P, NH * P], F32R, name="wg_t")
            nc.sync.dma_start(out=pwg[:], in_=wg[ii, :, :])
            pwi = wip.tile([P, NH * P], F32R, name="wi_t")
            nc.sync.dma_start(out=pwi[:], in_=wi[ii, :, :])
            preloaded[ii] = (pwg, pwi)

        # full-T router logits chunks, interleaved into the expert schedule
        rf_remaining = list(range(T // NC2))

        def emit_rf_chunk():
            if not rf_remaining:
                return
            tcx = rf_remaining.pop(0)
            ps_lt = psT.tile([E, NC2], F32, name="ps_lt")
            for hh in range(NH):
                rf = rfp.tile([P, NC2], F32R, name="rf")
                nc.sync.dma_start(
                    out=rf[:],
                    in_=flatT[hh, :, tcx * NC2:(tcx + 1) * NC2])
                nc.tensor.matmul(
                    ps_lt[:], lhsT=rkh_sb[:, hh, :], rhs=rf[:],
                    start=(hh == 0), stop=(hh == NH - 1))
            lt_sb = ltp.tile([E, NC2], F32, name="lt_sb")
            nc.vector.tensor_copy(lt_sb[:], ps_lt[:])
            nc.sync.dma_start(
                out=logitsT_out[:, tcx * NC2:(tcx + 1) * NC2], in_=lt_sb[:])

        t0g = 0
        for blk, TBg in enumerate(blocks):
            NTg = TBg // P

            # load activations in 512-token chunks so the router can start
            # after the first chunk column instead of the full block
            ft = []
            for hh in range(NH):
                f_t = flatp.tile([P, TB], F32R, name="ft")
                ft.append(f_t)
            for tcx in range(TBg // NC2):
                for hh in range(NH):
                    nc.sync.dma_start(
                        out=ft[hh][:, tcx * NC2:(tcx + 1) * NC2],
                        in_=fg[hh, :, t0g + tcx * NC2:t0g + (tcx + 1) * NC2])

            # gathered-token router: compensated logits -> top-2 weights
            lts = []
            for tcx in range(TBg // NC2):
                sl = slice(tcx * NC2, (tcx + 1) * NC2)
                ps_lt = psT.tile([E, NC2], F32, name="ps_lt")
                for hh in range(NH):
                    fl = flop.tile([P, NC2], F32R, name="fl")
                    nc.sync.dma_start(
                        out=fl[:],
                        in_=fg_lo[hh, :, t0g + tcx * NC2:
                                  t0g + (tcx + 1) * NC2])
                    nc.tensor.matmul(
                        ps_lt[:], lhsT=rkh_sb[:, hh, :], rhs=ft[hh][:, sl],
                        start=(hh == 0), stop=False)
                    nc.tensor.matmul(
                        ps_lt[:], lhsT=rkl_sb[:, hh, :], rhs=ft[hh][:, sl],
                        start=False, stop=False)
                    nc.tensor.matmul(
                        ps_lt[:], lhsT=rkh_sb[:, hh, :], rhs=fl[:],
                        start=False, stop=(hh == NH - 1))
                lt_sb = ltp.tile([E, NC2], F32, name="lt_sb")
                nc.vector.tensor_copy(lt_sb[:], ps_lt[:])
                lts.append(lt_sb)

            for tt in range(NTg):
                g_tt = (t0g + tt * P) // P
                tcx, off = divmod(tt * P, NC2)
                ps_r = psr.tile([P, E], F32, name="ps_r")
                nc.tensor.transpose(
                    ps_r[:], lts[tcx][:, off:off + P], ident[:E, :E])
                L = rtp.tile([P, E], F32, name="L")
                nc.vector.tensor_copy(L[:], ps_r[:])
                m1 = rtp.tile([P, 1], F32, name="m1")
                nc.vector.tensor_reduce(m1[:], L[:], axis=AX.X, op=OP.max)
                is1 = rtp.tile([P, E], F32, name="is1")
                nc.vector.tensor_scalar(
                    out=is1[:], in0=L[:], scalar1=m1[:], scalar2=None,
                    op0=OP.is_ge)
                big = rtp.tile([P, E], F32, name="big")
                nc.vector.tensor_scalar_mul(big[:], is1[:], -1e30)
                Lm = rtp.tile([P, E], F32, name="Lm")
                nc.vector.tensor_add(Lm[:], L[:], big[:])
                m2 = rtp.tile([P, 1], F32, name="m2")
                nc.vector.tensor_reduce(m2[:], Lm[:], axis=AX.X, op=OP.max)
                d12 = rtp.tile([P, 1], F32, name="d12")
                nc.vector.tensor_sub(d12[:], m1[:], m2[:])
                s1 = rtp.tile([P, 1], F32, name="s1")
                nc.scalar.activation(s1[:], d12[:], AF.Sigmoid)
                s2 = rtp.tile([P, 1], F32, name="s2")
                nc.vector.tensor_scalar(
                    out=s2[:], in0=s1[:], scalar1=-1.0, scalar2=1.0,
                    op0=OP.mult, op1=OP.add)
                sel2 = rtp.tile([P, E], F32, name="sel2")
                nc.vector.tensor_scalar(
                    out=sel2[:], in0=L[:], scalar1=m2[:], scalar2=None,
                    op0=OP.is_ge)
                is2 = rtp.tile([P, E], F32, name="is2")
                nc.vector.tensor_sub(is2[:], sel2[:], is1[:])
                w1 = rtp.tile([P, E], F32, name="w1")
                nc.vector.tensor_scalar_mul(w1[:], is1[:], s1[:])
                wr = rtp.tile([P, E], F32, name="wr")
                nc.vector.scalar_tensor_tensor(
                    out=wr[:], in0=is2[:], scalar=s2[:], in1=w1[:],
                    op0=OP.mult, op1=OP.add)
                wsel = rtp.tile([P, E], F32, name="wsel")
                nc.vector.tensor_mul(wsel[:], wr[:], sel_sb[:])
                nc.vector.tensor_reduce(
                    rsc[:, g_tt:g_tt + 1], wsel[:], axis=AX.X, op=OP.add)

            emit_rf_chunk()

            acc = {}
            for half in range(2):
                i_lo, i_hi = half * (NI // 2), (half + 1) * (NI // 2)

                hid = {}
                for ii in range(i_lo, i_hi):
                    if blk == 0 and ii in (4, 12):
                        emit_rf_chunk()
                    if blk == 0 and half == 0 and ii in preloaded:
                        wg_t, wi_t = preloaded[ii]
                    else:
                        wg_t = wgp.tile([P, NH * P], F32R, name="wg_t")
                        nc.sync.dma_start(out=wg_t[:], in_=wg[ii, :, :])
                        wi_t = wip.tile([P, NH * P], F32R, name="wi_t")
                        nc.sync.dma_start(out=wi_t[:], in_=wi[ii, :, :])
                    hid_t = hidp.tile([P, TB], F32R, name="hid_t")
                    for tcx in range(TBg // NC2):
                        sl = slice(tcx * NC2, (tcx + 1) * NC2)
                        ps_g = psg.tile([P, NC2], F32, name="ps_g")
                        ps_v = psv.tile([P, NC2], F32, name="ps_v")
                        for hh in range(NH):
                            nc.tensor.matmul(
                                ps_g[:],
                                lhsT=wg_t[:, hh * P:(hh + 1) * P],
                                rhs=ft[hh][:, sl],
                                start=(hh == 0), stop=(hh == NH - 1))
                        for hh in range(NH):
                            nc.tensor.matmul(
                                ps_v[:],
                                lhsT=wi_t[:, hh * P:(hh + 1) * P],
                                rhs=ft[hh][:, sl],
                                start=(hh == 0), stop=(hh == NH - 1))
                        si_t = evp.tile([P, NC2], F32, name="si_t")
                        nc.scalar.activation(si_t[:], ps_g[:], AF.Silu)
                        nc.vector.tensor_mul(hid_t[:, sl], si_t[:], ps_v[:])
                    hid[ii] = hid_t

                for ii in range(i_lo, i_hi):
                    wo_t = wop.tile([P, H], F32R, name="wo_t")
                    nc.sync.dma_start(out=wo_t[:], in_=wo[ii, :, :])
                    hid[ii] = (hid[ii], wo_t)

                for tt in range(NTg):
                    g_tt = (t0g + tt * P) // P
                    tsl = slice(tt * P, (tt + 1) * P)
                    for hcx in range(H // NC2):
                        hsl = slice(hcx * NC2, (hcx + 1) * NC2)
                        ps_o = pso.tile([P, NC2], F32, name="ps_o")
                        for ii in range(i_lo, i_hi):
                            hid_t, wo_t = hid[ii]
                            nc.tensor.matmul(
                                ps_o[:],
                                lhsT=hid_t[:, tsl],
                                rhs=wo_t[:, hsl],
                                start=(ii == i_lo), stop=(ii == i_hi - 1))
                        if half == 0:
                            a_t = accp.tile([P, NC2], F32, name="a_t")
                            nc.vector.tensor_scalar_mul(
                                a_t[:], ps_o[:], rsc[:, g_tt:g_tt + 1])
                            acc[(tt, hcx)] = a_t
                        else:
                            o_t = outp.tile([P, NC2], F32, name="o_t")
                            nc.vector.scalar_tensor_tensor(
                                out=o_t[:], in0=ps_o[:],
                                scalar=rsc[:, g_tt:g_tt + 1],
                                in1=acc[(tt, hcx)][:],
                                op0=OP.mult, op1=OP.add)
                            nc.sync.dma_start(
                                out=out[t0g + tt * P: t0g + (tt + 1) * P,
                                        hsl],
                                in_=o_t[:])
                emit_rf_chunk()
            t0g += TBg
        while rf_remaining:
            emit_rf_chunk()
    nc.finalize()
    return nc


def _fp22_split(x):
    """x -> (hi, lo): hi = x truncated to 13 mantissa bits (exact FP22),
    lo = x - hi (exact in fp32)."""
    hi = (x.view(np.uint32) & np.uint32(0xFFFFE000)).view(np.float32)
    return hi, x - hi


def _prep_in_maps(hidden_states, router_kernel, w_gate, w_inter, w_out):
    f32 = np.float32
    flat = np.ascontiguousarray(hidden_states, dtype=f32).reshape(T, H)
    flatT_full = np.ascontiguousarray(flat.T)
    flatT, flatT_lo = _fp22_split(flatT_full)
    flatT = flatT.reshape(NH, P, T)
    flatT_lo = np.ascontiguousarray(flatT_lo).reshape(NH, P, T)
    rk = np.ascontiguousarray(router_kernel, dtype=f32)
    rk_h, rk_l = _fp22_split(rk)
    rk_h = rk_h.reshape(NH, P, E)
    rk_l = np.ascontiguousarray(rk_l).reshape(NH, P, E)
    in_maps = []
    for c in range(NCORES):
        wg_c = np.ascontiguousarray(
            np.asarray(w_gate[c], dtype=f32).reshape(NH, P, NI, P)
            .transpose(2, 1, 0, 3).reshape(NI, P, NH * P))
        wi_c = np.ascontiguousarray(
            np.asarray(w_inter[c], dtype=f32).reshape(NH, P, NI, P)
            .transpose(2, 1, 0, 3).reshape(NI, P, NH * P))
        wo_c = np.ascontiguousarray(
            np.asarray(w_out[c], dtype=f32).reshape(NI, P, H))
        sel_c = np.zeros((P, E), dtype=f32)
        sel_c[:, c] = 1.0
        in_maps.append({
            "flatT": flatT, "flatT_lo": flatT_lo, "rk_h": rk_h, "rk_l": rk_l,
            "wg": wg_c, "wi": wi_c, "wo": wo_c, "sel": sel_c,
        })
    return in_maps


def _expert_weight_maps(w_gate, w_inter, w_out):
    f32 = np.float32
    maps = []
    for c in range(NCORES):
        wg_c = np.ascontiguousarray(
            np.asarray(w_gate[c], dtype=f32).reshape(NH, P, NI, P)
            .transpose(2, 1, 0, 3).reshape(NI, P, NH * P))
        wi_c = np.ascontiguousarray(
            np.asarray(w_inter[c], dtype=f32).reshape(NH, P, NI, P)
            .transpose(2, 1, 0, 3).reshape(NI, P, NH * P))
        wo_c = np.ascontiguousarray(
            np.asarray(w_out[c], dtype=f32).reshape(NI, P, H))
        sel_c = np.zeros((P, E), dtype=f32)
        sel_c[:, c] = 1.0
        maps.append({"wg": wg_c, "wi": wi_c, "wo": wo_c, "sel": sel_c})
    return maps


def run_on_device(hidden_states, router_kernel, w_gate, w_inter, w_out,
                  trace=False, force_dense=False, **trace_kw):
    """Shard + run SPMD on 8 cores; returns (out, logits, results)."""
    from concourse.bass_utils import run_bass_kernel_spmd

    f32 = np.float32
    flat = np.ascontiguousarray(hidden_states, dtype=f32).reshape(T, H)
    rk = np.ascontiguousarray(router_kernel, dtype=f32)
    rk_h, rk_l = _fp22_split(rk)
    rk_h = rk_h.reshape(NH, P, E)
    rk_l = np.ascontiguousarray(rk_l).reshape(NH, P, E)

    # host-side dispatch (sharding decision): exact top-2 per token
    logits64 = flat.astype(np.float64) @ rk.astype(np.float64)
    srt = np.sort(logits64, -1)
    margin = (srt[:, -2] - srt[:, -3]).min()
    thr = srt[:, -2:-1]                      # 2nd-largest logit per token
    sel_mask = logits64 >= thr               # [T, E] top-2 membership
    counts = sel_mask.sum(0)
    cap_raw = int(counts.max())
    use_sparse = (not force_dense) and margin > 1e-5 and cap_raw <= 3584

    wmaps = _expert_weight_maps(w_gate, w_inter, w_out)

    if use_sparse:
        cap = max(512, ((cap_raw + 511) // 512) * 512)
        blocks = [1024] * (cap // 1024) + ([512] if cap % 1024 else [])
        key = ("sparse", cap)
        if key not in _CACHE:
            _CACHE[key] = _build_nc_sparse(blocks)
        nc = _CACHE[key]

        flatT_full = np.ascontiguousarray(flat.T).reshape(NH, P, T)
        idxs, in_maps = [], []
        for c in range(NCORES):
            idx = np.nonzero(sel_mask[:, c])[0]
            idxs.append(idx)
            gath = np.zeros((cap, H), f32)
            gath[:len(idx)] = flat[idx]
            gT = np.ascontiguousarray(gath.T)
            g_hi, g_lo = _fp22_split(gT)
            in_maps.append({
                "flatT": flatT_full,
                "fg": g_hi.reshape(NH, P, cap),
                "fg_lo": np.ascontiguousarray(g_lo).reshape(NH, P, cap),
                "rk_h": rk_h, "rk_l": rk_l, **wmaps[c],
            })
        res = run_bass_kernel_spmd(
            nc, in_maps, core_ids=list(range(NCORES)), trace=trace,
            **trace_kw)
        parts = res.results
        out = np.zeros((T, H), f32)
        for c in range(NCORES):
            n = len(idxs[c])
            out[idxs[c]] += parts[c]["out"][:n]
        logits = np.ascontiguousarray(parts[0]["logitsT"].T)
    else:
        key = ("dense",)
        if key not in _CACHE:
            _CACHE[key] = _build_nc()
        nc = _CACHE[key]
        flatT_full, flatT_lo = _fp22_split(np.ascontiguousarray(flat.T))
        in_maps = []
        for c in range(NCORES):
            in_maps.append({
                "flatT": flatT_full.reshape(NH, P, T),
                "flatT_lo": np.ascontiguousarray(flatT_lo).reshape(NH, P, T),
                "rk_h": rk_h, "rk_l": rk_l, **wmaps[c],
            })
        res = run_bass_kernel_spmd(
            nc, in_maps, core_ids=list(range(NCORES)), trace=trace,
            **trace_kw)
        parts = res.results
        out = parts[0]["out"].astype(np.float32, copy=True)
        for c in range(1, NCORES):
            out += parts[c]["out"]
        logits = parts[0]["logits"]
    return out.reshape(2, 2048, H), logits, res


def kernel(hidden_states, router_kernel, w_gate, w_inter, w_out):
    out, logits, _ = run_on_device(
        hidden_states, router_kernel, w_gate, w_inter, w_out)
    return out, logits


# revision 25
# speedup vs baseline: 28532.5966x; 1.0099x over previous
"""Trainium2 Bass kernel: Mixtral-style MoE block (8 experts, top-2 router).

Sharding: expert-parallel across 8 NeuronCores — core c owns expert c's
w_gate/w_inter/w_out. The router is replicated (every core computes logits +
top-2 weights and scales its own expert's output by its routing weight);
the weighted expert sum is realized as a host-side sum over the 8 per-core
partial outputs at gather time.

Per-core compute layout (all matmuls via PE in float32r = full rate):
  flatT [H, T] (host-pretransposed)  ->  gateT/interT tiles [I_tile, T]
  hiddenT = silu(gateT) * interT     ->  out [T, H] = hiddenT.T @ w_out
The second matmul uses hiddenT tiles directly as the stationary operand, so
no on-device transposes are needed anywhere.
"""

import numpy as np

# ---- problem constants (hardcoded per contract) ----
P = 128
T = 4096           # tokens (B*S = 2*2048)
H = 1024           # hidden
ID = 2048          # intermediate
E = 8              # experts
NH = H // P        # 8 h-tiles
NI = ID // P       # 16 i-tiles
NB = 4             # token blocks
TB = T // NB       # 1024 tokens per block
NT = TB // P       # 8 t-tiles per block
NC2 = 512          # matmul moving free-dim chunk
NCORES = 8

_CACHE = {}


def _build_nc():
    import concourse.bass as bass
    import concourse.bacc as bacc
    from concourse import mybir
    from concourse import tile
    from concourse import masks

    F32 = mybir.dt.float32
    F32R = mybir.dt.float32r
    AF = mybir.ActivationFunctionType
    OP = mybir.AluOpType
    AX = mybir.AxisListType

    nc = bacc.Bacc(None, target_bir_lowering=False)

    # DRAM I/O (host-pretiled layouts; see kernel() below)
    # flatT carries the FP22-truncated "hi" part; flatT_lo the residual, so
    # the router can form an exactly-compensated fp32 logit product.
    flatT = nc.dram_tensor("flatT", [NH, P, T], F32R, kind="ExternalInput")
    flatT_lo = nc.dram_tensor("flatT_lo", [NH, P, T], F32R, kind="ExternalInput")
    rk_h = nc.dram_tensor("rk_h", [NH, P, E], F32R, kind="ExternalInput")
    rk_l = nc.dram_tensor("rk_l", [NH, P, E], F32R, kind="ExternalInput")
    wg = nc.dram_tensor("wg", [NI, P, NH * P], F32R, kind="ExternalInput")
    wi = nc.dram_tensor("wi", [NI, P, NH * P], F32R, kind="ExternalInput")
    wo = nc.dram_tensor("wo", [NI, P, H], F32R, kind="ExternalInput")
    sel = nc.dram_tensor("sel", [P, E], F32, kind="ExternalInput")
    out = nc.dram_tensor("out", [T, H], F32, kind="ExternalOutput")
    logits_out = nc.dram_tensor("logits", [T, E], F32, kind="ExternalOutput")

    from contextlib import ExitStack
    with ExitStack() as st:
            tc = st.enter_context(tile.TileContext(nc))
            pool = lambda name, bufs, **kw: st.enter_context(
                tc.tile_pool(name=name, bufs=bufs, **kw))
            consts = pool("consts", 1)
            flatp = pool("flatp", 9)
            wgp = pool("wgp", 3)
            wip = pool("wip", 3)
            wop = pool("wop", 10)
            hidp = pool("hidp", 9)
            evp = pool("evp", 4)
            accp = pool("accp", 17)
            outp = pool("outp", 4)
            rtp = pool("rtp", 4)
            flop = pool("flop", 3)
            ltp = pool("ltp", 3)
            psg = pool("psg", 2, space="PSUM")
            psv = pool("psv", 2, space="PSUM")
            pso = pool("pso", 2, space="PSUM")
            psT = pool("psT", 1, space="PSUM")
            psr = pool("psr", 1, space="PSUM")

            # ---- constants ----
            rkh_sb = consts.tile([P, NH, E], F32R)
            nc.sync.dma_start(out=rkh_sb[:], in_=rk_h[:].rearrange("h p e -> p h e"))
            rkl_sb = consts.tile([P, NH, E], F32R)
            nc.sync.dma_start(out=rkl_sb[:], in_=rk_l[:].rearrange("h p e -> p h e"))
            ident = consts.tile([P, P], F32)
            masks.make_identity(nc, ident[:])
            sel_sb = consts.tile([P, E], F32)
            nc.sync.dma_start(out=sel_sb[:], in_=sel[:])
            # per-token routing scalar for this core's expert, one column per
            # global t-tile
            rsc = consts.tile([P, NB * NT], F32)

            for blk in range(NB):
                t0 = blk * TB

                # ---- load this block's activation tiles [P(h), TB] ----
                ft = []
                for hh in range(NH):
                    f_t = flatp.tile([P, TB], F32R, name="ft")
                    nc.sync.dma_start(out=f_t[:], in_=flatT[hh, :, t0:t0 + TB])
                    ft.append(f_t)

                # ---- router logits, [E, t] layout, hi/lo-compensated ----
                # logits = hi@rk_h + hi@rk_l + lo@rk_h  (error ~2^-28)
                lts = []
                for tcx in range(TB // NC2):
                    sl = slice(tcx * NC2, (tcx + 1) * NC2)
                    ps_lt = psT.tile([E, NC2], F32, name="ps_lt")
                    for hh in range(NH):
                        fl = flop.tile([P, NC2], F32R, name="fl")
                        nc.sync.dma_start(
                            out=fl[:],
                            in_=flatT_lo[hh, :, t0 + tcx * NC2:
                                         t0 + (tcx + 1) * NC2])
                        nc.tensor.matmul(
                            ps_lt[:], lhsT=rkh_sb[:, hh, :], rhs=ft[hh][:, sl],
                            start=(hh == 0), stop=False)
                        nc.tensor.matmul(
                            ps_lt[:], lhsT=rkl_sb[:, hh, :], rhs=ft[hh][:, sl],
                            start=False, stop=False)
                        nc.tensor.matmul(
                            ps_lt[:], lhsT=rkh_sb[:, hh, :], rhs=fl[:],
                            start=False, stop=(hh == NH - 1))
                    lt_sb = ltp.tile([E, NC2], F32, name="lt_sb")
                    nc.vector.tensor_copy(lt_sb[:], ps_lt[:])
                    lts.append(lt_sb)

                for tt in range(NT):
                    g_tt = blk * NT + tt
                    tcx, off = divmod(tt * P, NC2)
                    ps_r = psr.tile([P, E], F32, name="ps_r")
                    nc.tensor.transpose(
                        ps_r[:], lts[tcx][:, off:off + P], ident[:E, :E])
                    L = rtp.tile([P, E], F32, name="L")
                    nc.vector.tensor_copy(L[:], ps_r[:])
                    nc.sync.dma_start(
                        out=logits_out[t0 + tt * P: t0 + (tt + 1) * P, :],
                        in_=L[:],
                    )
                    # top-2 + renormalize:  w1 = sigmoid(l1-l2), w2 = 1-w1
                    m1 = rtp.tile([P, 1], F32, name="m1")
                    nc.vector.tensor_reduce(m1[:], L[:], axis=AX.X, op=OP.max)
                    is1 = rtp.tile([P, E], F32, name="is1")
                    nc.vector.tensor_scalar(
                        out=is1[:], in0=L[:], scalar1=m1[:], scalar2=None,
                        op0=OP.is_ge,
                    )
                    big = rtp.tile([P, E], F32, name="big")
                    nc.vector.tensor_scalar_mul(big[:], is1[:], -1e30)
                    Lm = rtp.tile([P, E], F32, name="Lm")
                    nc.vector.tensor_add(Lm[:], L[:], big[:])
                    m2 = rtp.tile([P, 1], F32, name="m2")
                    nc.vector.tensor_reduce(m2[:], Lm[:], axis=AX.X, op=OP.max)
                    d12 = rtp.tile([P, 1], F32, name="d12")
                    nc.vector.tensor_sub(d12[:], m1[:], m2[:])
                    s1 = rtp.tile([P, 1], F32, name="s1")
                    nc.scalar.activation(s1[:], d12[:], AF.Sigmoid)
                    s2 = rtp.tile([P, 1], F32, name="s2")
                    nc.vector.tensor_scalar(
                        out=s2[:], in0=s1[:], scalar1=-1.0, scalar2=1.0,
                        op0=OP.mult, op1=OP.add,
                    )
                    sel2 = rtp.tile([P, E], F32, name="sel2")
                    nc.vector.tensor_scalar(
                        out=sel2[:], in0=L[:], scalar1=m2[:], scalar2=None,
                        op0=OP.is_ge,
                    )
                    is2 = rtp.tile([P, E], F32, name="is2")
                    nc.vector.tensor_sub(is2[:], sel2[:], is1[:])
                    w1 = rtp.tile([P, E], F32, name="w1")
                    nc.vector.tensor_scalar_mul(w1[:], is1[:], s1[:])
                    wr = rtp.tile([P, E], F32, name="wr")
                    nc.vector.scalar_tensor_tensor(
                        out=wr[:], in0=is2[:], scalar=s2[:], in1=w1[:],
                        op0=OP.mult, op1=OP.add,
                    )
                    wsel = rtp.tile([P, E], F32, name="wsel")
                    nc.vector.tensor_mul(wsel[:], wr[:], sel_sb[:])
                    nc.vector.tensor_reduce(
                        rsc[:, g_tt:g_tt + 1], wsel[:], axis=AX.X, op=OP.add,
                    )

                # ---- two half-sweeps over I: SwiGLU then down-proj partial ----
                acc = {}
                for half in range(2):
                    i_lo, i_hi = half * (NI // 2), (half + 1) * (NI // 2)

                    # phase 1: hiddenT tiles [P(i), TB] for this half
                    hid = {}
                    for ii in range(i_lo, i_hi):
                        wg_t = wgp.tile([P, NH * P], F32R, name="wg_t")
                        nc.sync.dma_start(out=wg_t[:], in_=wg[ii, :, :])
                        wi_t = wip.tile([P, NH * P], F32R, name="wi_t")
                        nc.sync.dma_start(out=wi_t[:], in_=wi[ii, :, :])
                        hid_t = hidp.tile([P, TB], F32R, name="hid_t")
                        for tcx in range(TB // NC2):
                            sl = slice(tcx * NC2, (tcx + 1) * NC2)
                            ps_g = psg.tile([P, NC2], F32, name="ps_g")
                            ps_v = psv.tile([P, NC2], F32, name="ps_v")
                            for hh in range(NH):
                                nc.tensor.matmul(
                                    ps_g[:],
                                    lhsT=wg_t[:, hh * P:(hh + 1) * P],
                                    rhs=ft[hh][:, sl],
                                    start=(hh == 0), stop=(hh == NH - 1),
                                )
                            for hh in range(NH):
                                nc.tensor.matmul(
                                    ps_v[:],
                                    lhsT=wi_t[:, hh * P:(hh + 1) * P],
                                    rhs=ft[hh][:, sl],
                                    start=(hh == 0), stop=(hh == NH - 1),
                                )
                            si_t = evp.tile([P, NC2], F32, name="si_t")
                            nc.scalar.activation(si_t[:], ps_g[:], AF.Silu)
                            nc.vector.tensor_mul(hid_t[:, sl], si_t[:], ps_v[:])
                        hid[ii] = hid_t

                    # phase 2: out[t,h] partial over this I-half
                    for ii in range(i_lo, i_hi):
                        wo_t = wop.tile([P, H], F32R, name="wo_t")
                        nc.sync.dma_start(out=wo_t[:], in_=wo[ii, :, :])
                        hid[ii] = (hid[ii], wo_t)

                    for tt in range(NT):
                        g_tt = blk * NT + tt
                        tsl = slice(tt * P, (tt + 1) * P)
                        for hcx in range(H // NC2):
                            hsl = slice(hcx * NC2, (hcx + 1) * NC2)
                            ps_o = pso.tile([P, NC2], F32, name="ps_o")
                            for ii in range(i_lo, i_hi):
                                hid_t, wo_t = hid[ii]
                                nc.tensor.matmul(
                                    ps_o[:],
                                    lhsT=hid_t[:, tsl],
                                    rhs=wo_t[:, hsl],
                                    start=(ii == i_lo), stop=(ii == i_hi - 1),
                                )
                            if half == 0:
                                # stash routing-scaled partial in SBUF
                                a_t = accp.tile([P, NC2], F32, name="a_t")
                                nc.vector.tensor_scalar_mul(
                                    a_t[:], ps_o[:], rsc[:, g_tt:g_tt + 1],
                                )
                                acc[(tt, hcx)] = a_t
                            else:
                                o_t = outp.tile([P, NC2], F32, name="o_t")
                                nc.vector.scalar_tensor_tensor(
                                    out=o_t[:], in0=ps_o[:],
                                    scalar=rsc[:, g_tt:g_tt + 1],
                                    in1=acc[(tt, hcx)][:],
                                    op0=OP.mult, op1=OP.add,
                                )
                                nc.sync.dma_start(
                                    out=out[t0 + tt * P: t0 + (tt + 1) * P, hsl],
                                    in_=o_t[:],
                                )
    nc.finalize()
    return nc


def _build_nc_sparse(blocks):
    """Capacity-gathered variant: expert FFN computed only for this core's
    assigned tokens (host gathers per-expert token sets; zero-weight tokens
    contribute exactly 0 in the reference sum, so skipping them is exact).

    blocks: token-block sizes for the gathered dimension, e.g. [1024, 512]
    for capacity 1536. Full-T router logits are still computed on device
    ([E, T] layout, host transposes); per-gathered-token top-2 weights are
    computed on device from compensated logits.
    """
    import concourse.bass as bass
    import concourse.bacc as bacc
    from concourse import mybir
    from concourse import tile
    from concourse import masks
    from contextlib import ExitStack

    F32 = mybir.dt.float32
    F32R = mybir.dt.float32r
    AF = mybir.ActivationFunctionType
    OP = mybir.AluOpType
    AX = mybir.AxisListType

    CAP = sum(blocks)
    nc = bacc.Bacc(None, target_bir_lowering=False)

    flatT = nc.dram_tensor("flatT", [NH, P, T], F32R, kind="ExternalInput")
    fg = nc.dram_tensor("fg", [NH, P, CAP], F32R, kind="ExternalInput")
    fg_lo = nc.dram_tensor("fg_lo", [NH, P, CAP], F32R, kind="ExternalInput")
    rk_h = nc.dram_tensor("rk_h", [NH, P, E], F32R, kind="ExternalInput")
    rk_l = nc.dram_tensor("rk_l", [NH, P, E], F32R, kind="ExternalInput")
    wg = nc.dram_tensor("wg", [NI, P, NH * P], F32R, kind="ExternalInput")
    wi = nc.dram_tensor("wi", [NI, P, NH * P], F32R, kind="ExternalInput")
    wo = nc.dram_tensor("wo", [NI, P, H], F32R, kind="ExternalInput")
    sel = nc.dram_tensor("sel", [P, E], F32, kind="ExternalInput")
    out = nc.dram_tensor("out", [CAP, H], F32, kind="ExternalOutput")
    logitsT_out = nc.dram_tensor("logitsT", [E, T], F32, kind="ExternalOutput")

    with ExitStack() as st:
        tc = st.enter_context(tile.TileContext(nc))
        pool = lambda name, bufs, **kw: st.enter_context(
            tc.tile_pool(name=name, bufs=bufs, **kw))
        consts = pool("consts", 1)
        flatp = pool("flatp", 9)
        wgp = pool("wgp", 3)
        wip = pool("wip", 3)
        wop = pool("wop", 10)
        hidp = pool("hidp", 9)
        evp = pool("evp", 3)
        accp = pool("accp", 17)
        outp = pool("outp", 4)
        rtp = pool("rtp", 4)
        flop = pool("flop", 3)
        ltp = pool("ltp", 3)
        rfp = pool("rfp", 3)
        psg = pool("psg", 2, space="PSUM")
        psv = pool("psv", 2, space="PSUM")
        pso = pool("pso", 2, space="PSUM")
        psT = pool("psT", 1, space="PSUM")
        psr = pool("psr", 1, space="PSUM")

        rkh_sb = consts.tile([P, NH, E], F32R)
        nc.sync.dma_start(out=rkh_sb[:], in_=rk_h[:].rearrange("h p e -> p h e"))
        rkl_sb = consts.tile([P, NH, E], F32R)
        nc.sync.dma_start(out=rkl_sb[:], in_=rk_l[:].rearrange("h p e -> p h e"))
        ident = consts.tile([P, P], F32)
        masks.make_identity(nc, ident[:])
        sel_sb = consts.tile([P, E], F32)
        nc.sync.dma_start(out=sel_sb[:], in_=sel[:])
        rsc = consts.tile([P, 32], F32)

        preloaded = {}

        # full-T router logits chunks, interleaved into the expert schedule
        rf_remaining = list(range(T // NC2))

        def emit_rf_chunk():
            if not rf_remaining:
                return
            tcx = rf_remaining.pop(0)
            ps_lt = psT.tile([E, NC2], F32, name="ps_lt")
            for hh in range(NH):
                rf = rfp.tile([P, NC2], F32R, name="rf")
                nc.sync.dma_start(
                    out=rf[:],
                    in_=flatT[hh, :, tcx * NC2:(tcx + 1) * NC2])
                nc.tensor.matmul(
                    ps_lt[:], lhsT=rkh_sb[:, hh, :], rhs=rf[:],
                    start=(hh == 0), stop=(hh == NH - 1))
            lt_sb = ltp.tile([E, NC2], F32, name="lt_sb")
            nc.vector.tensor_copy(lt_sb[:], ps_lt[:])
            nc.sync.dma_start(
                out=logitsT_out[:, tcx * NC2:(tcx + 1) * NC2], in_=lt_sb[:])

        t0g = 0
        for blk, TBg in enumerate(blocks):
            NTg = TBg // P

            # load activations in 512-token chunks so the router can start
            # after the first chunk column instead of the full block
            ft = []
            for hh in range(NH):
                f_t = flatp.tile([P, TB], F32R, name="ft")
                ft.append(f_t)
            for tcx in range(TBg // NC2):
                for hh in range(NH):
                    nc.sync.dma_start(
                        out=ft[hh][:, tcx * NC2:(tcx + 1) * NC2],
                        in_=fg[hh, :, t0g + tcx * NC2:t0g + (tcx + 1) * NC2])
                if blk == 0 and tcx == 0:
                    # prefetch the first expert weight tiles behind the first
                    # activation chunk so phase 1 starts right after the
                    # router without delaying the router itself
                    for ii in range(2):
                        pwg = wgp.tile([P, NH * P], F32R, name="wg_t")
                        nc.sync.dma_start(out=pwg[:], in_=wg[ii, :, :])
                        pwi = wip.tile([P, NH * P], F32R, name="wi_t")
                        nc.sync.dma_start(out=pwi[:], in_=wi[ii, :, :])
                        preloaded[ii] = (pwg, pwi)

            # gathered-token router: compensated logits -> top-2 weights
            lts = []
            for tcx in range(TBg // NC2):
                sl = slice(tcx * NC2, (tcx + 1) * NC2)
                ps_lt = psT.tile([E, NC2], F32, name="ps_lt")
                for hh in range(NH):
                    fl = flop.tile([P, NC2], F32R, name="fl")
                    nc.sync.dma_start(
                        out=fl[:],
                        in_=fg_lo[hh, :, t0g + tcx * NC2:
                                  t0g + (tcx + 1) * NC2])
                    nc.tensor.matmul(
                        ps_lt[:], lhsT=rkh_sb[:, hh, :], rhs=ft[hh][:, sl],
                        start=(hh == 0), stop=False)
                    nc.tensor.matmul(
                        ps_lt[:], lhsT=rkl_sb[:, hh, :], rhs=ft[hh][:, sl],
                        start=False, stop=False)
                    nc.tensor.matmul(
                        ps_lt[:], lhsT=rkh_sb[:, hh, :], rhs=fl[:],
                        start=False, stop=(hh == NH - 1))
                lt_sb = ltp.tile([E, NC2], F32, name="lt_sb")
                nc.vector.tensor_copy(lt_sb[:], ps_lt[:])
                lts.append(lt_sb)

            for tt in range(NTg):
                g_tt = (t0g + tt * P) // P
                tcx, off = divmod(tt * P, NC2)
                ps_r = psr.tile([P, E], F32, name="ps_r")
                nc.tensor.transpose(
                    ps_r[:], lts[tcx][:, off:off + P], ident[:E, :E])
                L = rtp.tile([P, E], F32, name="L")
                nc.vector.tensor_copy(L[:], ps_r[:])
                m1 = rtp.tile([P, 1], F32, name="m1")
                nc.vector.tensor_reduce(m1[:], L[:], axis=AX.X, op=OP.max)
                is1 = rtp.tile([P, E], F32, name="is1")
                nc.vector.tensor_scalar(
                    out=is1[:], in0=L[:], scalar1=m1[:], scalar2=None,
                    op0=OP.is_ge)
                big = rtp.tile([P, E], F32, name="big")
                nc.vector.tensor_scalar_mul(big[:], is1[:], -1e30)
                Lm = rtp.tile([P, E], F32, name="Lm")
                nc.vector.tensor_add(Lm[:], L[:], big[:])
                m2 = rtp.tile([P, 1], F32, name="m2")
                nc.vector.tensor_reduce(m2[:], Lm[:], axis=AX.X, op=OP.max)
                d12 = rtp.tile([P, 1], F32, name="d12")
                nc.vector.tensor_sub(d12[:], m1[:], m2[:])
                s1 = rtp.tile([P, 1], F32, name="s1")
                nc.scalar.activation(s1[:], d12[:], AF.Sigmoid)
                s2 = rtp.tile([P, 1], F32, name="s2")
                nc.vector.tensor_scalar(
                    out=s2[:], in0=s1[:], scalar1=-1.0, scalar2=1.0,
                    op0=OP.mult, op1=OP.add)
                sel2 = rtp.tile([P, E], F32, name="sel2")
                nc.vector.tensor_scalar(
                    out=sel2[:], in0=L[:], scalar1=m2[:], scalar2=None,
                    op0=OP.is_ge)
                is2 = rtp.tile([P, E], F32, name="is2")
                nc.vector.tensor_sub(is2[:], sel2[:], is1[:])
                w1 = rtp.tile([P, E], F32, name="w1")
                nc.vector.tensor_scalar_mul(w1[:], is1[:], s1[:])
                wr = rtp.tile([P, E], F32, name="wr")
                nc.vector.scalar_tensor_tensor(
                    out=wr[:], in0=is2[:], scalar=s2[:], in1=w1[:],
                    op0=OP.mult, op1=OP.add)
                wsel = rtp.tile([P, E], F32, name="wsel")
                nc.vector.tensor_mul(wsel[:], wr[:], sel_sb[:])
                nc.vector.tensor_reduce(
                    rsc[:, g_tt:g_tt + 1], wsel[:], axis=AX.X, op=OP.add)

            emit_rf_chunk()

            acc = {}
            for half in range(2):
                i_lo, i_hi = half * (NI // 2), (half + 1) * (NI // 2)

                hid = {}
                for ii in range(i_lo, i_hi):
                    if blk == 0 and ii in (2, 5, 10, 13):
                        emit_rf_chunk()
                    if blk == 0 and half == 0 and ii in preloaded:
                        wg_t, wi_t = preloaded[ii]
                    else:
                        wg_t = wgp.tile([P, NH * P], F32R, name="wg_t")
                        nc.sync.dma_start(out=wg_t[:], in_=wg[ii, :, :])
                        wi_t = wip.tile([P, NH * P], F32R, name="wi_t")
                        nc.sync.dma_start(out=wi_t[:], in_=wi[ii, :, :])
                    hid_t = hidp.tile([P, TB], F32R, name="hid_t")
                    for tcx in range(TBg // NC2):
                        sl = slice(tcx * NC2, (tcx + 1) * NC2)
                        ps_g = psg.tile([P, NC2], F32, name="ps_g")
                        ps_v = psv.tile([P, NC2], F32, name="ps_v")
                        for hh in range(NH):
                            nc.tensor.matmul(
                                ps_g[:],
                                lhsT=wg_t[:, hh * P:(hh + 1) * P],
                                rhs=ft[hh][:, sl],
                                start=(hh == 0), stop=(hh == NH - 1))
                        for hh in range(NH):
                            nc.tensor.matmul(
                                ps_v[:],
                                lhsT=wi_t[:, hh * P:(hh + 1) * P],
                                rhs=ft[hh][:, sl],
                                start=(hh == 0), stop=(hh == NH - 1))
                        si_t = evp.tile([P, NC2], F32, name="si_t")
                        nc.scalar.activation(si_t[:], ps_g[:], AF.Silu)
                        nc.vector.tensor_mul(hid_t[:, sl], si_t[:], ps_v[:])
                    hid[ii] = hid_t

                for ii in range(i_lo, i_hi):
                    wo_t = wop.tile([P, H], F32R, name="wo_t")
                    nc.sync.dma_start(out=wo_t[:], in_=wo[ii, :, :])
                    hid[ii] = (hid[ii], wo_t)

                for tt in range(NTg):
                    g_tt = (t0g + tt * P) // P
                    tsl = slice(tt * P, (tt + 1) * P)
                    for hcx in range(H // NC2):
                        hsl = slice(hcx * NC2, (hcx + 1) * NC2)
                        ps_o = pso.tile([P, NC2], F32, name="ps_o")
                        for ii in range(i_lo, i_hi):
                            hid_t, wo_t = hid[ii]
                            nc.tensor.matmul(
                                ps_o[:],
                                lhsT=hid_t[:, tsl],
                                rhs=wo_t[:, hsl],
                                start=(ii == i_lo), stop=(ii == i_hi - 1))
                        if half == 0:
                            a_t = accp.tile([P, NC2], F32, name="a_t")
                            nc.vector.tensor_scalar_mul(
                                a_t[:], ps_o[:], rsc[:, g_tt:g_tt + 1])
                            acc[(tt, hcx)] = a_t
                        else:
                            o_t = outp.tile([P, NC2], F32, name="o_t")
                            nc.vector.scalar_tensor_tensor(
                                out=o_t[:], in0=ps_o[:],
                                scalar=rsc[:, g_tt:g_tt + 1],
                                in1=acc[(tt, hcx)][:],
                                op0=OP.mult, op1=OP.add)
                            nc.sync.dma_start(
                                out=out[t0g + tt * P: t0g + (tt + 1) * P,
                                        hsl],
                                in_=o_t[:])
                emit_rf_chunk()
            t0g += TBg
        while rf_remaining:
            emit_rf_chunk()
    nc.finalize()
    return nc


def _fp22_split(x):
    """x -> (hi, lo): hi = x truncated to 13 mantissa bits (exact FP22),
    lo = x - hi (exact in fp32)."""
    hi = (x.view(np.uint32) & np.uint32(0xFFFFE000)).view(np.float32)
    return hi, x - hi


def _prep_in_maps(hidden_states, router_kernel, w_gate, w_inter, w_out):
    f32 = np.float32
    flat = np.ascontiguousarray(hidden_states, dtype=f32).reshape(T, H)
    flatT_full = np.ascontiguousarray(flat.T)
    flatT, flatT_lo = _fp22_split(flatT_full)
    flatT = flatT.reshape(NH, P, T)
    flatT_lo = np.ascontiguousarray(flatT_lo).reshape(NH, P, T)
    rk = np.ascontiguousarray(router_kernel, dtype=f32)
    rk_h, rk_l = _fp22_split(rk)
    rk_h = rk_h.reshape(NH, P, E)
    rk_l = np.ascontiguousarray(rk_l).reshape(NH, P, E)
    in_maps = []
    for c in range(NCORES):
        wg_c = np.ascontiguousarray(
            np.asarray(w_gate[c], dtype=f32).reshape(NH, P, NI, P)
            .transpose(2, 1, 0, 3).reshape(NI, P, NH * P))
        wi_c = np.ascontiguousarray(
            np.asarray(w_inter[c], dtype=f32).reshape(NH, P, NI, P)
            .transpose(2, 1, 0, 3).reshape(NI, P, NH * P))
        wo_c = np.ascontiguousarray(
            np.asarray(w_out[c], dtype=f32).reshape(NI, P, H))
        sel_c = np.zeros((P, E), dtype=f32)
        sel_c[:, c] = 1.0
        in_maps.append({
            "flatT": flatT, "flatT_lo": flatT_lo, "rk_h": rk_h, "rk_l": rk_l,
            "wg": wg_c, "wi": wi_c, "wo": wo_c, "sel": sel_c,
        })
    return in_maps


def _expert_weight_maps(w_gate, w_inter, w_out):
    f32 = np.float32
    maps = []
    for c in range(NCORES):
        wg_c = np.ascontiguousarray(
            np.asarray(w_gate[c], dtype=f32).reshape(NH, P, NI, P)
            .transpose(2, 1, 0, 3).reshape(NI, P, NH * P))
        wi_c = np.ascontiguousarray(
            np.asarray(w_inter[c], dtype=f32).reshape(NH, P, NI, P)
            .transpose(2, 1, 0, 3).reshape(NI, P, NH * P))
        wo_c = np.ascontiguousarray(
            np.asarray(w_out[c], dtype=f32).reshape(NI, P, H))
        sel_c = np.zeros((P, E), dtype=f32)
        sel_c[:, c] = 1.0
        maps.append({"wg": wg_c, "wi": wi_c, "wo": wo_c, "sel": sel_c})
    return maps


def run_on_device(hidden_states, router_kernel, w_gate, w_inter, w_out,
                  trace=False, force_dense=False, **trace_kw):
    """Shard + run SPMD on 8 cores; returns (out, logits, results)."""
    from concourse.bass_utils import run_bass_kernel_spmd

    f32 = np.float32
    flat = np.ascontiguousarray(hidden_states, dtype=f32).reshape(T, H)
    rk = np.ascontiguousarray(router_kernel, dtype=f32)
    rk_h, rk_l = _fp22_split(rk)
    rk_h = rk_h.reshape(NH, P, E)
    rk_l = np.ascontiguousarray(rk_l).reshape(NH, P, E)

    # host-side dispatch (sharding decision): exact top-2 per token
    logits64 = flat.astype(np.float64) @ rk.astype(np.float64)
    srt = np.sort(logits64, -1)
    margin = (srt[:, -2] - srt[:, -3]).min()
    thr = srt[:, -2:-1]                      # 2nd-largest logit per token
    sel_mask = logits64 >= thr               # [T, E] top-2 membership
    counts = sel_mask.sum(0)
    cap_raw = int(counts.max())
    use_sparse = (not force_dense) and margin > 1e-5 and cap_raw <= 3584

    wmaps = _expert_weight_maps(w_gate, w_inter, w_out)

    if use_sparse:
        cap = max(512, ((cap_raw + 511) // 512) * 512)
        blocks = [1024] * (cap // 1024) + ([512] if cap % 1024 else [])
        key = ("sparse", cap)
        if key not in _CACHE:
            _CACHE[key] = _build_nc_sparse(blocks)
        nc = _CACHE[key]

        flatT_full = np.ascontiguousarray(flat.T).reshape(NH, P, T)
        idxs, in_maps = [], []
        for c in range(NCORES):
            idx = np.nonzero(sel_mask[:, c])[0]
            idxs.append(idx)
            gath = np.zeros((cap, H), f32)
            gath[:len(idx)] = flat[idx]
            gT = np.ascontiguousarray(gath.T)
            g_hi, g_lo = _fp22_split(gT)
            in_maps.append({
                "flatT": flatT_full,
                "fg": g_hi.reshape(NH, P, cap),
                "fg_lo": np.ascontiguousarray(g_lo).reshape(NH, P, cap),
                "rk_h": rk_h, "rk_l": rk_l, **wmaps[c],
            })
        res = run_bass_kernel_spmd(
            nc, in_maps, core_ids=list(range(NCORES)), trace=trace,
            **trace_kw)
        parts = res.results
        out = np.zeros((T, H), f32)
        for c in range(NCORES):
            n = len(idxs[c])
            out[idxs[c]] += parts[c]["out"][:n]
        logits = np.ascontiguousarray(parts[0]["logitsT"].T)
    else:
        key = ("dense",)
        if key not in _CACHE:
            _CACHE[key] = _build_nc()
        nc = _CACHE[key]
        flatT_full, flatT_lo = _fp22_split(np.ascontiguousarray(flat.T))
        in_maps = []
        for c in range(NCORES):
            in_maps.append({
                "flatT": flatT_full.reshape(NH, P, T),
                "flatT_lo": np.ascontiguousarray(flatT_lo).reshape(NH, P, T),
                "rk_h": rk_h, "rk_l": rk_l, **wmaps[c],
            })
        res = run_bass_kernel_spmd(
            nc, in_maps, core_ids=list(range(NCORES)), trace=trace,
            **trace_kw)
        parts = res.results
        out = parts[0]["out"].astype(np.float32, copy=True)
        for c in range(1, NCORES):
            out += parts[c]["out"]
        logits = parts[0]["logits"]
    return out.reshape(2, 2048, H), logits, res


def kernel(hidden_states, router_kernel, w_gate, w_inter, w_out):
    out, logits, _ = run_on_device(
        hidden_states, router_kernel, w_gate, w_inter, w_out)
    return out, logits


# revision 28
# speedup vs baseline: 30637.9242x; 1.0738x over previous
"""Trainium2 Bass kernel: Mixtral-style MoE block (8 experts, top-2 router).

Sharding: expert-parallel across 8 NeuronCores — core c owns expert c's
w_gate/w_inter/w_out. The router is replicated (every core computes logits +
top-2 weights and scales its own expert's output by its routing weight);
the weighted expert sum is realized as a host-side sum over the 8 per-core
partial outputs at gather time.

Per-core compute layout (all matmuls via PE in float32r = full rate):
  flatT [H, T] (host-pretransposed)  ->  gateT/interT tiles [I_tile, T]
  hiddenT = silu(gateT) * interT     ->  out [T, H] = hiddenT.T @ w_out
The second matmul uses hiddenT tiles directly as the stationary operand, so
no on-device transposes are needed anywhere.
"""

import numpy as np

# ---- problem constants (hardcoded per contract) ----
P = 128
T = 4096           # tokens (B*S = 2*2048)
H = 1024           # hidden
ID = 2048          # intermediate
E = 8              # experts
NH = H // P        # 8 h-tiles
NI = ID // P       # 16 i-tiles
NB = 4             # token blocks
TB = T // NB       # 1024 tokens per block
NT = TB // P       # 8 t-tiles per block
NC2 = 512          # matmul moving free-dim chunk
NCORES = 8

_CACHE = {}


def _build_nc():
    import concourse.bass as bass
    import concourse.bacc as bacc
    from concourse import mybir
    from concourse import tile
    from concourse import masks

    F32 = mybir.dt.float32
    F32R = mybir.dt.float32r
    AF = mybir.ActivationFunctionType
    OP = mybir.AluOpType
    AX = mybir.AxisListType

    nc = bacc.Bacc(None, target_bir_lowering=False)

    # DRAM I/O (host-pretiled layouts; see kernel() below)
    # flatT carries the FP22-truncated "hi" part; flatT_lo the residual, so
    # the router can form an exactly-compensated fp32 logit product.
    flatT = nc.dram_tensor("flatT", [NH, P, T], F32R, kind="ExternalInput")
    flatT_lo = nc.dram_tensor("flatT_lo", [NH, P, T], F32R, kind="ExternalInput")
    rk_h = nc.dram_tensor("rk_h", [NH, P, E], F32R, kind="ExternalInput")
    rk_l = nc.dram_tensor("rk_l", [NH, P, E], F32R, kind="ExternalInput")
    wg = nc.dram_tensor("wg", [NI, P, NH * P], F32R, kind="ExternalInput")
    wi = nc.dram_tensor("wi", [NI, P, NH * P], F32R, kind="ExternalInput")
    wo = nc.dram_tensor("wo", [NI, P, H], F32R, kind="ExternalInput")
    sel = nc.dram_tensor("sel", [P, E], F32, kind="ExternalInput")
    out = nc.dram_tensor("out", [T, H], F32, kind="ExternalOutput")
    logits_out = nc.dram_tensor("logits", [T, E], F32, kind="ExternalOutput")

    from contextlib import ExitStack
    with ExitStack() as st:
            tc = st.enter_context(tile.TileContext(nc))
            pool = lambda name, bufs, **kw: st.enter_context(
                tc.tile_pool(name=name, bufs=bufs, **kw))
            consts = pool("consts", 1)
            flatp = pool("flatp", 9)
            wgp = pool("wgp", 3)
            wip = pool("wip", 3)
            wop = pool("wop", 10)
            hidp = pool("hidp", 9)
            evp = pool("evp", 4)
            accp = pool("accp", 17)
            outp = pool("outp", 4)
            rtp = pool("rtp", 4)
            flop = pool("flop", 3)
            ltp = pool("ltp", 3)
            psg = pool("psg", 2, space="PSUM")
            psv = pool("psv", 2, space="PSUM")
            pso = pool("pso", 2, space="PSUM")
            psT = pool("psT", 1, space="PSUM")
            psr = pool("psr", 1, space="PSUM")

            # ---- constants ----
            rkh_sb = consts.tile([P, NH, E], F32R)
            nc.sync.dma_start(out=rkh_sb[:], in_=rk_h[:].rearrange("h p e -> p h e"))
            rkl_sb = consts.tile([P, NH, E], F32R)
            nc.sync.dma_start(out=rkl_sb[:], in_=rk_l[:].rearrange("h p e -> p h e"))
            ident = consts.tile([P, P], F32)
            masks.make_identity(nc, ident[:])
            sel_sb = consts.tile([P, E], F32)
            nc.sync.dma_start(out=sel_sb[:], in_=sel[:])
            # per-token routing scalar for this core's expert, one column per
            # global t-tile
            rsc = consts.tile([P, NB * NT], F32)

            for blk in range(NB):
                t0 = blk * TB

                # ---- load this block's activation tiles [P(h), TB] ----
                ft = []
                for hh in range(NH):
                    f_t = flatp.tile([P, TB], F32R, name="ft")
                    nc.sync.dma_start(out=f_t[:], in_=flatT[hh, :, t0:t0 + TB])
                    ft.append(f_t)

                # ---- router logits, [E, t] layout, hi/lo-compensated ----
                # logits = hi@rk_h + hi@rk_l + lo@rk_h  (error ~2^-28)
                lts = []
                for tcx in range(TB // NC2):
                    sl = slice(tcx * NC2, (tcx + 1) * NC2)
                    ps_lt = psT.tile([E, NC2], F32, name="ps_lt")
                    for hh in range(NH):
                        fl = flop.tile([P, NC2], F32R, name="fl")
                        nc.sync.dma_start(
                            out=fl[:],
                            in_=flatT_lo[hh, :, t0 + tcx * NC2:
                                         t0 + (tcx + 1) * NC2])
                        nc.tensor.matmul(
                            ps_lt[:], lhsT=rkh_sb[:, hh, :], rhs=ft[hh][:, sl],
                            start=(hh == 0), stop=False)
                        nc.tensor.matmul(
                            ps_lt[:], lhsT=rkl_sb[:, hh, :], rhs=ft[hh][:, sl],
                            start=False, stop=False)
                        nc.tensor.matmul(
                            ps_lt[:], lhsT=rkh_sb[:, hh, :], rhs=fl[:],
                            start=False, stop=(hh == NH - 1))
                    lt_sb = ltp.tile([E, NC2], F32, name="lt_sb")
                    nc.vector.tensor_copy(lt_sb[:], ps_lt[:])
                    lts.append(lt_sb)

                for tt in range(NT):
                    g_tt = blk * NT + tt
                    tcx, off = divmod(tt * P, NC2)
                    ps_r = psr.tile([P, E], F32, name="ps_r")
                    nc.tensor.transpose(
                        ps_r[:], lts[tcx][:, off:off + P], ident[:E, :E])
                    L = rtp.tile([P, E], F32, name="L")
                    nc.vector.tensor_copy(L[:], ps_r[:])
                    nc.sync.dma_start(
                        out=logits_out[t0 + tt * P: t0 + (tt + 1) * P, :],
                        in_=L[:],
                    )
                    # top-2 + renormalize:  w1 = sigmoid(l1-l2), w2 = 1-w1
                    m1 = rtp.tile([P, 1], F32, name="m1")
                    nc.vector.tensor_reduce(m1[:], L[:], axis=AX.X, op=OP.max)
                    is1 = rtp.tile([P, E], F32, name="is1")
                    nc.vector.tensor_scalar(
                        out=is1[:], in0=L[:], scalar1=m1[:], scalar2=None,
                        op0=OP.is_ge,
                    )
                    big = rtp.tile([P, E], F32, name="big")
                    nc.vector.tensor_scalar_mul(big[:], is1[:], -1e30)
                    Lm = rtp.tile([P, E], F32, name="Lm")
                    nc.vector.tensor_add(Lm[:], L[:], big[:])
                    m2 = rtp.tile([P, 1], F32, name="m2")
                    nc.vector.tensor_reduce(m2[:], Lm[:], axis=AX.X, op=OP.max)
                    d12 = rtp.tile([P, 1], F32, name="d12")
                    nc.vector.tensor_sub(d12[:], m1[:], m2[:])
                    s1 = rtp.tile([P, 1], F32, name="s1")
                    nc.scalar.activation(s1[:], d12[:], AF.Sigmoid)
                    s2 = rtp.tile([P, 1], F32, name="s2")
                    nc.vector.tensor_scalar(
                        out=s2[:], in0=s1[:], scalar1=-1.0, scalar2=1.0,
                        op0=OP.mult, op1=OP.add,
                    )
                    sel2 = rtp.tile([P, E], F32, name="sel2")
                    nc.vector.tensor_scalar(
                        out=sel2[:], in0=L[:], scalar1=m2[:], scalar2=None,
                        op0=OP.is_ge,
                    )
                    is2 = rtp.tile([P, E], F32, name="is2")
                    nc.vector.tensor_sub(is2[:], sel2[:], is1[:])
                    w1 = rtp.tile([P, E], F32, name="w1")
                    nc.vector.tensor_scalar_mul(w1[:], is1[:], s1[:])
                    wr = rtp.tile([P, E], F32, name="wr")
                    nc.vector.scalar_tensor_tensor(
                        out=wr[:], in0=is2[:], scalar=s2[:], in1=w1[:],
                        op0=OP.mult, op1=OP.add,
                    )
                    wsel = rtp.tile([P, E], F32, name="wsel")
                    nc.vector.tensor_mul(wsel[:], wr[:], sel_sb[:])
                    nc.vector.tensor_reduce(
                        rsc[:, g_tt:g_tt + 1], wsel[:], axis=AX.X, op=OP.add,
                    )

                # ---- two half-sweeps over I: SwiGLU then down-proj partial ----
                acc = {}
                for half in range(2):
                    i_lo, i_hi = half * (NI // 2), (half + 1) * (NI // 2)

                    # phase 1: hiddenT tiles [P(i), TB] for this half
                    hid = {}
                    for ii in range(i_lo, i_hi):
                        wg_t = wgp.tile([P, NH * P], F32R, name="wg_t")
                        nc.sync.dma_start(out=wg_t[:], in_=wg[ii, :, :])
                        wi_t = wip.tile([P, NH * P], F32R, name="wi_t")
                        nc.sync.dma_start(out=wi_t[:], in_=wi[ii, :, :])
                        hid_t = hidp.tile([P, TB], F32R, name="hid_t")
                        for tcx in range(TB // NC2):
                            sl = slice(tcx * NC2, (tcx + 1) * NC2)
                            ps_g = psg.tile([P, NC2], F32, name="ps_g")
                            ps_v = psv.tile([P, NC2], F32, name="ps_v")
                            for hh in range(NH):
                                nc.tensor.matmul(
                                    ps_g[:],
                                    lhsT=wg_t[:, hh * P:(hh + 1) * P],
                                    rhs=ft[hh][:, sl],
                                    start=(hh == 0), stop=(hh == NH - 1),
                                )
                            for hh in range(NH):
                                nc.tensor.matmul(
                                    ps_v[:],
                                    lhsT=wi_t[:, hh * P:(hh + 1) * P],
                                    rhs=ft[hh][:, sl],
                                    start=(hh == 0), stop=(hh == NH - 1),
                                )
                            si_t = evp.tile([P, NC2], F32, name="si_t")
                            nc.scalar.activation(si_t[:], ps_g[:], AF.Silu)
                            nc.vector.tensor_mul(hid_t[:, sl], si_t[:], ps_v[:])
                        hid[ii] = hid_t

                    # phase 2: out[t,h] partial over this I-half
                    for ii in range(i_lo, i_hi):
                        wo_t = wop.tile([P, H], F32R, name="wo_t")
                        nc.sync.dma_start(out=wo_t[:], in_=wo[ii, :, :])
                        hid[ii] = (hid[ii], wo_t)

                    for tt in range(NT):
                        g_tt = blk * NT + tt
                        tsl = slice(tt * P, (tt + 1) * P)
                        for hcx in range(H // NC2):
                            hsl = slice(hcx * NC2, (hcx + 1) * NC2)
                            ps_o = pso.tile([P, NC2], F32, name="ps_o")
                            for ii in range(i_lo, i_hi):
                                hid_t, wo_t = hid[ii]
                                nc.tensor.matmul(
                                    ps_o[:],
                                    lhsT=hid_t[:, tsl],
                                    rhs=wo_t[:, hsl],
                                    start=(ii == i_lo), stop=(ii == i_hi - 1),
                                )
                            if half == 0:
                                # stash routing-scaled partial in SBUF
                                a_t = accp.tile([P, NC2], F32, name="a_t")
                                nc.vector.tensor_scalar_mul(
                                    a_t[:], ps_o[:], rsc[:, g_tt:g_tt + 1],
                                )
                                acc[(tt, hcx)] = a_t
                            else:
                                o_t = outp.tile([P, NC2], F32, name="o_t")
                                nc.vector.scalar_tensor_tensor(
                                    out=o_t[:], in0=ps_o[:],
                                    scalar=rsc[:, g_tt:g_tt + 1],
                                    in1=acc[(tt, hcx)][:],
                                    op0=OP.mult, op1=OP.add,
                                )
                                nc.sync.dma_start(
                                    out=out[t0 + tt * P: t0 + (tt + 1) * P, hsl],
                                    in_=o_t[:],
                                )
    nc.finalize()
    return nc


def _build_nc_sparse(blocks):
    """Capacity-gathered variant: expert FFN computed only for this core's
    assigned tokens (host gathers per-expert token sets; zero-weight tokens
    contribute exactly 0 in the reference sum, so skipping them is exact).

    blocks: token-block sizes for the gathered dimension, e.g. [1024, 512]
    for capacity 1536. Full-T router logits are still computed on device
    ([E, T] layout, host transposes); per-gathered-token top-2 weights are
    computed on device from compensated logits.
    """
    import concourse.bass as bass
    import concourse.bacc as bacc
    from concourse import mybir
    from concourse import tile
    from concourse import masks
    from contextlib import ExitStack

    F32 = mybir.dt.float32
    F32R = mybir.dt.float32r
    AF = mybir.ActivationFunctionType
    OP = mybir.AluOpType
    AX = mybir.AxisListType

    CAP = sum(blocks)
    nc = bacc.Bacc(None, target_bir_lowering=False)

    flatT = nc.dram_tensor("flatT", [NH, P, T], F32R, kind="ExternalInput")
    fg = nc.dram_tensor("fg", [NH, P, CAP], F32R, kind="ExternalInput")
    fg_lo = nc.dram_tensor("fg_lo", [NH, P, CAP], F32R, kind="ExternalInput")
    rk_h = nc.dram_tensor("rk_h", [NH, P, E], F32R, kind="ExternalInput")
    rk_l = nc.dram_tensor("rk_l", [NH, P, E], F32R, kind="ExternalInput")
    wg = nc.dram_tensor("wg", [NI, P, NH * P], F32R, kind="ExternalInput")
    wi = nc.dram_tensor("wi", [NI, P, NH * P], F32R, kind="ExternalInput")
    wo = nc.dram_tensor("wo", [NI, P, H], F32R, kind="ExternalInput")
    sel = nc.dram_tensor("sel", [P, E], F32, kind="ExternalInput")
    out = nc.dram_tensor("out", [CAP, H], F32, kind="ExternalOutput")
    logitsT_out = nc.dram_tensor("logitsT", [E, T], F32, kind="ExternalOutput")

    with ExitStack() as st:
        tc = st.enter_context(tile.TileContext(nc))
        pool = lambda name, bufs, **kw: st.enter_context(
            tc.tile_pool(name=name, bufs=bufs, **kw))
        consts = pool("consts", 1)
        flatp = pool("flatp", 9)
        wgp = pool("wgp", 3)
        wip = pool("wip", 3)
        wop = pool("wop", 10)
        hidp = pool("hidp", 9)
        evp = pool("evp", 3)
        accp = pool("accp", 17)
        outp = pool("outp", 4)
        rtp = pool("rtp", 4)
        flop = pool("flop", 3)
        ltp = pool("ltp", 3)
        rfp = pool("rfp", 3)
        psg = pool("psg", 2, space="PSUM")
        psv = pool("psv", 2, space="PSUM")
        pso = pool("pso", 2, space="PSUM")
        psT = pool("psT", 1, space="PSUM")
        psr = pool("psr", 1, space="PSUM")

        rkh_sb = consts.tile([P, NH, E], F32R)
        nc.sync.dma_start(out=rkh_sb[:], in_=rk_h[:].rearrange("h p e -> p h e"))
        rkl_sb = consts.tile([P, NH, E], F32R)
        nc.sync.dma_start(out=rkl_sb[:], in_=rk_l[:].rearrange("h p e -> p h e"))
        ident = consts.tile([P, P], F32)
        masks.make_identity(nc, ident[:])
        sel_sb = consts.tile([P, E], F32)
        nc.sync.dma_start(out=sel_sb[:], in_=sel[:])
        rsc = consts.tile([P, 32], F32)

        preloaded = {}

        # full-T router logits chunks, interleaved into the expert schedule
        rf_remaining = list(range(T // NC2))

        def emit_rf_chunk():
            if not rf_remaining:
                return
            tcx = rf_remaining.pop(0)
            ps_lt = psT.tile([E, NC2], F32, name="ps_lt")
            for hh in range(NH):
                rf = rfp.tile([P, NC2], F32R, name="rf")
                nc.sync.dma_start(
                    out=rf[:],
                    in_=flatT[hh, :, tcx * NC2:(tcx + 1) * NC2])
                nc.tensor.matmul(
                    ps_lt[:], lhsT=rkh_sb[:, hh, :], rhs=rf[:],
                    start=(hh == 0), stop=(hh == NH - 1))
            lt_sb = ltp.tile([E, NC2], F32, name="lt_sb")
            nc.vector.tensor_copy(lt_sb[:], ps_lt[:])
            nc.sync.dma_start(
                out=logitsT_out[:, tcx * NC2:(tcx + 1) * NC2], in_=lt_sb[:])

        t0g = 0
        for blk, TBg in enumerate(blocks):
            NTg = TBg // P

            # load activations in 512-token chunks so the router can start
            # after the first chunk column instead of the full block
            ft = []
            for hh in range(NH):
                f_t = flatp.tile([P, TB], F32R, name="ft")
                ft.append(f_t)
            chunks = [(o, min(NC2, TBg - o)) for o in range(0, TBg, NC2)]
            for (co, cw) in chunks:
                for hh in range(NH):
                    nc.sync.dma_start(
                        out=ft[hh][:, co:co + cw],
                        in_=fg[hh, :, t0g + co:t0g + co + cw])
                if blk == 0 and co == 0:
                    # prefetch the first expert weight tiles behind the first
                    # activation chunk so phase 1 starts right after the
                    # router without delaying the router itself
                    for ii in range(2):
                        pwg = wgp.tile([P, NH * P], F32R, name="wg_t")
                        nc.sync.dma_start(out=pwg[:], in_=wg[ii, :, :])
                        pwi = wip.tile([P, NH * P], F32R, name="wi_t")
                        nc.sync.dma_start(out=pwi[:], in_=wi[ii, :, :])
                        preloaded[ii] = (pwg, pwi)

            # gathered-token router: compensated logits -> top-2 weights
            lts = []
            for (co, cw) in chunks:
                sl = slice(co, co + cw)
                ps_lt = psT.tile([E, NC2], F32, name="ps_lt")
                for hh in range(NH):
                    fl = flop.tile([P, NC2], F32R, name="fl")
                    nc.sync.dma_start(
                        out=fl[:, :cw],
                        in_=fg_lo[hh, :, t0g + co:t0g + co + cw])
                    nc.tensor.matmul(
                        ps_lt[:, :cw], lhsT=rkh_sb[:, hh, :],
                        rhs=ft[hh][:, sl], start=(hh == 0), stop=False)
                    nc.tensor.matmul(
                        ps_lt[:, :cw], lhsT=rkl_sb[:, hh, :],
                        rhs=ft[hh][:, sl], start=False, stop=False)
                    nc.tensor.matmul(
                        ps_lt[:, :cw], lhsT=rkh_sb[:, hh, :], rhs=fl[:, :cw],
                        start=False, stop=(hh == NH - 1))
                lt_sb = ltp.tile([E, NC2], F32, name="lt_sb")
                nc.vector.tensor_copy(lt_sb[:, :cw], ps_lt[:, :cw])
                lts.append(lt_sb)

            for tt in range(NTg):
                g_tt = (t0g + tt * P) // P
                tcx, off = divmod(tt * P, NC2)
                ps_r = psr.tile([P, E], F32, name="ps_r")
                nc.tensor.transpose(
                    ps_r[:], lts[tcx][:, off:off + P], ident[:E, :E])
                L = rtp.tile([P, E], F32, name="L")
                nc.vector.tensor_copy(L[:], ps_r[:])
                m1 = rtp.tile([P, 1], F32, name="m1")
                nc.vector.tensor_reduce(m1[:], L[:], axis=AX.X, op=OP.max)
                is1 = rtp.tile([P, E], F32, name="is1")
                nc.vector.tensor_scalar(
                    out=is1[:], in0=L[:], scalar1=m1[:], scalar2=None,
                    op0=OP.is_ge)
                big = rtp.tile([P, E], F32, name="big")
                nc.vector.tensor_scalar_mul(big[:], is1[:], -1e30)
                Lm = rtp.tile([P, E], F32, name="Lm")
                nc.vector.tensor_add(Lm[:], L[:], big[:])
                m2 = rtp.tile([P, 1], F32, name="m2")
                nc.vector.tensor_reduce(m2[:], Lm[:], axis=AX.X, op=OP.max)
                d12 = rtp.tile([P, 1], F32, name="d12")
                nc.vector.tensor_sub(d12[:], m1[:], m2[:])
                s1 = rtp.tile([P, 1], F32, name="s1")
                nc.scalar.activation(s1[:], d12[:], AF.Sigmoid)
                s2 = rtp.tile([P, 1], F32, name="s2")
                nc.vector.tensor_scalar(
                    out=s2[:], in0=s1[:], scalar1=-1.0, scalar2=1.0,
                    op0=OP.mult, op1=OP.add)
                sel2 = rtp.tile([P, E], F32, name="sel2")
                nc.vector.tensor_scalar(
                    out=sel2[:], in0=L[:], scalar1=m2[:], scalar2=None,
                    op0=OP.is_ge)
                is2 = rtp.tile([P, E], F32, name="is2")
                nc.vector.tensor_sub(is2[:], sel2[:], is1[:])
                w1 = rtp.tile([P, E], F32, name="w1")
                nc.vector.tensor_scalar_mul(w1[:], is1[:], s1[:])
                wr = rtp.tile([P, E], F32, name="wr")
                nc.vector.scalar_tensor_tensor(
                    out=wr[:], in0=is2[:], scalar=s2[:], in1=w1[:],
                    op0=OP.mult, op1=OP.add)
                wsel = rtp.tile([P, E], F32, name="wsel")
                nc.vector.tensor_mul(wsel[:], wr[:], sel_sb[:])
                nc.vector.tensor_reduce(
                    rsc[:, g_tt:g_tt + 1], wsel[:], axis=AX.X, op=OP.add)

            emit_rf_chunk()

            acc = {}
            for half in range(2):
                i_lo, i_hi = half * (NI // 2), (half + 1) * (NI // 2)

                hid = {}
                for ii in range(i_lo, i_hi):
                    if blk == 0 and ii in (2, 5, 10, 13):
                        emit_rf_chunk()
                    if blk == 0 and half == 0 and ii in preloaded:
                        wg_t, wi_t = preloaded[ii]
                    else:
                        wg_t = wgp.tile([P, NH * P], F32R, name="wg_t")
                        nc.sync.dma_start(out=wg_t[:], in_=wg[ii, :, :])
                        wi_t = wip.tile([P, NH * P], F32R, name="wi_t")
                        nc.sync.dma_start(out=wi_t[:], in_=wi[ii, :, :])
                    hid_t = hidp.tile([P, TB], F32R, name="hid_t")
                    for (co, cw) in chunks:
                        sl = slice(co, co + cw)
                        ps_g = psg.tile([P, NC2], F32, name="ps_g")
                        ps_v = psv.tile([P, NC2], F32, name="ps_v")
                        for hh in range(NH):
                            nc.tensor.matmul(
                                ps_g[:, :cw],
                                lhsT=wg_t[:, hh * P:(hh + 1) * P],
                                rhs=ft[hh][:, sl],
                                start=(hh == 0), stop=(hh == NH - 1))
                        for hh in range(NH):
                            nc.tensor.matmul(
                                ps_v[:, :cw],
                                lhsT=wi_t[:, hh * P:(hh + 1) * P],
                                rhs=ft[hh][:, sl],
                                start=(hh == 0), stop=(hh == NH - 1))
                        si_t = evp.tile([P, NC2], F32, name="si_t")
                        nc.scalar.activation(si_t[:, :cw], ps_g[:, :cw], AF.Silu)
                        nc.vector.tensor_mul(
                            hid_t[:, sl], si_t[:, :cw], ps_v[:, :cw])
                    hid[ii] = hid_t

                for ii in range(i_lo, i_hi):
                    wo_t = wop.tile([P, H], F32R, name="wo_t")
                    nc.sync.dma_start(out=wo_t[:], in_=wo[ii, :, :])
                    hid[ii] = (hid[ii], wo_t)

                for tt in range(NTg):
                    g_tt = (t0g + tt * P) // P
                    tsl = slice(tt * P, (tt + 1) * P)
                    for hcx in range(H // NC2):
                        hsl = slice(hcx * NC2, (hcx + 1) * NC2)
                        ps_o = pso.tile([P, NC2], F32, name="ps_o")
                        for ii in range(i_lo, i_hi):
                            hid_t, wo_t = hid[ii]
                            nc.tensor.matmul(
                                ps_o[:],
                                lhsT=hid_t[:, tsl],
                                rhs=wo_t[:, hsl],
                                start=(ii == i_lo), stop=(ii == i_hi - 1))
                        if half == 0:
                            a_t = accp.tile([P, NC2], F32, name="a_t")
                            nc.vector.tensor_scalar_mul(
                                a_t[:], ps_o[:], rsc[:, g_tt:g_tt + 1])
                            acc[(tt, hcx)] = a_t
                        else:
                            o_t = outp.tile([P, NC2], F32, name="o_t")
                            nc.vector.scalar_tensor_tensor(
                                out=o_t[:], in0=ps_o[:],
                                scalar=rsc[:, g_tt:g_tt + 1],
                                in1=acc[(tt, hcx)][:],
                                op0=OP.mult, op1=OP.add)
                            nc.sync.dma_start(
                                out=out[t0g + tt * P: t0g + (tt + 1) * P,
                                        hsl],
                                in_=o_t[:])
                emit_rf_chunk()
            t0g += TBg
        while rf_remaining:
            emit_rf_chunk()
    nc.finalize()
    return nc


def _fp22_split(x):
    """x -> (hi, lo): hi = x truncated to 13 mantissa bits (exact FP22),
    lo = x - hi (exact in fp32)."""
    hi = (x.view(np.uint32) & np.uint32(0xFFFFE000)).view(np.float32)
    return hi, x - hi


def _prep_in_maps(hidden_states, router_kernel, w_gate, w_inter, w_out):
    f32 = np.float32
    flat = np.ascontiguousarray(hidden_states, dtype=f32).reshape(T, H)
    flatT_full = np.ascontiguousarray(flat.T)
    flatT, flatT_lo = _fp22_split(flatT_full)
    flatT = flatT.reshape(NH, P, T)
    flatT_lo = np.ascontiguousarray(flatT_lo).reshape(NH, P, T)
    rk = np.ascontiguousarray(router_kernel, dtype=f32)
    rk_h, rk_l = _fp22_split(rk)
    rk_h = rk_h.reshape(NH, P, E)
    rk_l = np.ascontiguousarray(rk_l).reshape(NH, P, E)
    in_maps = []
    for c in range(NCORES):
        wg_c = np.ascontiguousarray(
            np.asarray(w_gate[c], dtype=f32).reshape(NH, P, NI, P)
            .transpose(2, 1, 0, 3).reshape(NI, P, NH * P))
        wi_c = np.ascontiguousarray(
            np.asarray(w_inter[c], dtype=f32).reshape(NH, P, NI, P)
            .transpose(2, 1, 0, 3).reshape(NI, P, NH * P))
        wo_c = np.ascontiguousarray(
            np.asarray(w_out[c], dtype=f32).reshape(NI, P, H))
        sel_c = np.zeros((P, E), dtype=f32)
        sel_c[:, c] = 1.0
        in_maps.append({
            "flatT": flatT, "flatT_lo": flatT_lo, "rk_h": rk_h, "rk_l": rk_l,
            "wg": wg_c, "wi": wi_c, "wo": wo_c, "sel": sel_c,
        })
    return in_maps


def _expert_weight_maps(w_gate, w_inter, w_out):
    f32 = np.float32
    maps = []
    for c in range(NCORES):
        wg_c = np.ascontiguousarray(
            np.asarray(w_gate[c], dtype=f32).reshape(NH, P, NI, P)
            .transpose(2, 1, 0, 3).reshape(NI, P, NH * P))
        wi_c = np.ascontiguousarray(
            np.asarray(w_inter[c], dtype=f32).reshape(NH, P, NI, P)
            .transpose(2, 1, 0, 3).reshape(NI, P, NH * P))
        wo_c = np.ascontiguousarray(
            np.asarray(w_out[c], dtype=f32).reshape(NI, P, H))
        sel_c = np.zeros((P, E), dtype=f32)
        sel_c[:, c] = 1.0
        maps.append({"wg": wg_c, "wi": wi_c, "wo": wo_c, "sel": sel_c})
    return maps


def run_on_device(hidden_states, router_kernel, w_gate, w_inter, w_out,
                  trace=False, force_dense=False, **trace_kw):
    """Shard + run SPMD on 8 cores; returns (out, logits, results)."""
    from concourse.bass_utils import run_bass_kernel_spmd

    f32 = np.float32
    flat = np.ascontiguousarray(hidden_states, dtype=f32).reshape(T, H)
    rk = np.ascontiguousarray(router_kernel, dtype=f32)
    rk_h, rk_l = _fp22_split(rk)
    rk_h = rk_h.reshape(NH, P, E)
    rk_l = np.ascontiguousarray(rk_l).reshape(NH, P, E)

    # host-side dispatch (sharding decision): exact top-2 per token
    logits64 = flat.astype(np.float64) @ rk.astype(np.float64)
    srt = np.sort(logits64, -1)
    margin = (srt[:, -2] - srt[:, -3]).min()
    thr = srt[:, -2:-1]                      # 2nd-largest logit per token
    sel_mask = logits64 >= thr               # [T, E] top-2 membership
    counts = sel_mask.sum(0)
    cap_raw = int(counts.max())
    use_sparse = (not force_dense) and margin > 1e-5 and cap_raw <= 3584

    wmaps = _expert_weight_maps(w_gate, w_inter, w_out)

    if use_sparse:
        cap = max(512, ((cap_raw + 255) // 256) * 256)
        blocks = [1024] * (cap // 1024)
        if cap % 1024:
            blocks.append(cap % 1024)
        key = ("sparse", cap)
        if key not in _CACHE:
            _CACHE[key] = _build_nc_sparse(blocks)
        nc = _CACHE[key]

        flatT_full = np.ascontiguousarray(flat.T).reshape(NH, P, T)
        idxs, in_maps = [], []
        for c in range(NCORES):
            idx = np.nonzero(sel_mask[:, c])[0]
            idxs.append(idx)
            gath = np.zeros((cap, H), f32)
            gath[:len(idx)] = flat[idx]
            gT = np.ascontiguousarray(gath.T)
            g_hi, g_lo = _fp22_split(gT)
            in_maps.append({
                "flatT": flatT_full,
                "fg": g_hi.reshape(NH, P, cap),
                "fg_lo": np.ascontiguousarray(g_lo).reshape(NH, P, cap),
                "rk_h": rk_h, "rk_l": rk_l, **wmaps[c],
            })
        res = run_bass_kernel_spmd(
            nc, in_maps, core_ids=list(range(NCORES)), trace=trace,
            **trace_kw)
        parts = res.results
        out = np.zeros((T, H), f32)
        for c in range(NCORES):
            n = len(idxs[c])
            out[idxs[c]] += parts[c]["out"][:n]
        logits = np.ascontiguousarray(parts[0]["logitsT"].T)
    else:
        key = ("dense",)
        if key not in _CACHE:
            _CACHE[key] = _build_nc()
        nc = _CACHE[key]
        flatT_full, flatT_lo = _fp22_split(np.ascontiguousarray(flat.T))
        in_maps = []
        for c in range(NCORES):
            in_maps.append({
                "flatT": flatT_full.reshape(NH, P, T),
                "flatT_lo": np.ascontiguousarray(flatT_lo).reshape(NH, P, T),
                "rk_h": rk_h, "rk_l": rk_l, **wmaps[c],
            })
        res = run_bass_kernel_spmd(
            nc, in_maps, core_ids=list(range(NCORES)), trace=trace,
            **trace_kw)
        parts = res.results
        out = parts[0]["out"].astype(np.float32, copy=True)
        for c in range(1, NCORES):
            out += parts[c]["out"]
        logits = parts[0]["logits"]
    return out.reshape(2, 2048, H), logits, res


def kernel(hidden_states, router_kernel, w_gate, w_inter, w_out):
    out, logits, _ = run_on_device(
        hidden_states, router_kernel, w_gate, w_inter, w_out)
    return out, logits


# revision 29
# speedup vs baseline: 31091.3427x; 1.0148x over previous
"""Trainium2 Bass kernel: Mixtral-style MoE block (8 experts, top-2 router).

Sharding: expert-parallel across 8 NeuronCores — core c owns expert c's
w_gate/w_inter/w_out. The router is replicated (every core computes logits +
top-2 weights and scales its own expert's output by its routing weight);
the weighted expert sum is realized as a host-side sum over the 8 per-core
partial outputs at gather time.

Per-core compute layout (all matmuls via PE in float32r = full rate):
  flatT [H, T] (host-pretransposed)  ->  gateT/interT tiles [I_tile, T]
  hiddenT = silu(gateT) * interT     ->  out [T, H] = hiddenT.T @ w_out
The second matmul uses hiddenT tiles directly as the stationary operand, so
no on-device transposes are needed anywhere.
"""

import numpy as np

# ---- problem constants (hardcoded per contract) ----
P = 128
T = 4096           # tokens (B*S = 2*2048)
H = 1024           # hidden
ID = 2048          # intermediate
E = 8              # experts
NH = H // P        # 8 h-tiles
NI = ID // P       # 16 i-tiles
NB = 4             # token blocks
TB = T // NB       # 1024 tokens per block
NT = TB // P       # 8 t-tiles per block
NC2 = 512          # matmul moving free-dim chunk
NCORES = 8

_CACHE = {}


def _build_nc():
    import concourse.bass as bass
    import concourse.bacc as bacc
    from concourse import mybir
    from concourse import tile
    from concourse import masks

    F32 = mybir.dt.float32
    F32R = mybir.dt.float32r
    AF = mybir.ActivationFunctionType
    OP = mybir.AluOpType
    AX = mybir.AxisListType

    nc = bacc.Bacc(None, target_bir_lowering=False)

    # DRAM I/O (host-pretiled layouts; see kernel() below)
    # flatT carries the FP22-truncated "hi" part; flatT_lo the residual, so
    # the router can form an exactly-compensated fp32 logit product.
    flatT = nc.dram_tensor("flatT", [NH, P, T], F32R, kind="ExternalInput")
    flatT_lo = nc.dram_tensor("flatT_lo", [NH, P, T], F32R, kind="ExternalInput")
    rk_h = nc.dram_tensor("rk_h", [NH, P, E], F32R, kind="ExternalInput")
    rk_l = nc.dram_tensor("rk_l", [NH, P, E], F32R, kind="ExternalInput")
    wg = nc.dram_tensor("wg", [NI, P, NH * P], F32R, kind="ExternalInput")
    wi = nc.dram_tensor("wi", [NI, P, NH * P], F32R, kind="ExternalInput")
    wo = nc.dram_tensor("wo", [NI, P, H], F32R, kind="ExternalInput")
    sel = nc.dram_tensor("sel", [P, E], F32, kind="ExternalInput")
    out = nc.dram_tensor("out", [T, H], F32, kind="ExternalOutput")
    logits_out = nc.dram_tensor("logits", [T, E], F32, kind="ExternalOutput")

    from contextlib import ExitStack
    with ExitStack() as st:
            tc = st.enter_context(tile.TileContext(nc))
            pool = lambda name, bufs, **kw: st.enter_context(
                tc.tile_pool(name=name, bufs=bufs, **kw))
            consts = pool("consts", 1)
            flatp = pool("flatp", 9)
            wgp = pool("wgp", 3)
            wip = pool("wip", 3)
            wop = pool("wop", 10)
            hidp = pool("hidp", 9)
            evp = pool("evp", 4)
            accp = pool("accp", 17)
            outp = pool("outp", 4)
            rtp = pool("rtp", 4)
            flop = pool("flop", 3)
            ltp = pool("ltp", 3)
            psg = pool("psg", 2, space="PSUM")
            psv = pool("psv", 2, space="PSUM")
            pso = pool("pso", 2, space="PSUM")
            psT = pool("psT", 1, space="PSUM")
            psr = pool("psr", 1, space="PSUM")

            # ---- constants ----
            rkh_sb = consts.tile([P, NH, E], F32R)
            nc.sync.dma_start(out=rkh_sb[:], in_=rk_h[:].rearrange("h p e -> p h e"))
            rkl_sb = consts.tile([P, NH, E], F32R)
            nc.sync.dma_start(out=rkl_sb[:], in_=rk_l[:].rearrange("h p e -> p h e"))
            ident = consts.tile([P, P], F32)
            masks.make_identity(nc, ident[:])
            sel_sb = consts.tile([P, E], F32)
            nc.sync.dma_start(out=sel_sb[:], in_=sel[:])
            # per-token routing scalar for this core's expert, one column per
            # global t-tile
            rsc = consts.tile([P, NB * NT], F32)

            for blk in range(NB):
                t0 = blk * TB

                # ---- load this block's activation tiles [P(h), TB] ----
                ft = []
                for hh in range(NH):
                    f_t = flatp.tile([P, TB], F32R, name="ft")
                    nc.sync.dma_start(out=f_t[:], in_=flatT[hh, :, t0:t0 + TB])
                    ft.append(f_t)

                # ---- router logits, [E, t] layout, hi/lo-compensated ----
                # logits = hi@rk_h + hi@rk_l + lo@rk_h  (error ~2^-28)
                lts = []
                for tcx in range(TB // NC2):
                    sl = slice(tcx * NC2, (tcx + 1) * NC2)
                    ps_lt = psT.tile([E, NC2], F32, name="ps_lt")
                    for hh in range(NH):
                        fl = flop.tile([P, NC2], F32R, name="fl")
                        nc.sync.dma_start(
                            out=fl[:],
                            in_=flatT_lo[hh, :, t0 + tcx * NC2:
                                         t0 + (tcx + 1) * NC2])
                        nc.tensor.matmul(
                            ps_lt[:], lhsT=rkh_sb[:, hh, :], rhs=ft[hh][:, sl],
                            start=(hh == 0), stop=False)
                        nc.tensor.matmul(
                            ps_lt[:], lhsT=rkl_sb[:, hh, :], rhs=ft[hh][:, sl],
                            start=False, stop=False)
                        nc.tensor.matmul(
                            ps_lt[:], lhsT=rkh_sb[:, hh, :], rhs=fl[:],
                            start=False, stop=(hh == NH - 1))
                    lt_sb = ltp.tile([E, NC2], F32, name="lt_sb")
                    nc.vector.tensor_copy(lt_sb[:], ps_lt[:])
                    lts.append(lt_sb)

                for tt in range(NT):
                    g_tt = blk * NT + tt
                    tcx, off = divmod(tt * P, NC2)
                    ps_r = psr.tile([P, E], F32, name="ps_r")
                    nc.tensor.transpose(
                        ps_r[:], lts[tcx][:, off:off + P], ident[:E, :E])
                    L = rtp.tile([P, E], F32, name="L")
                    nc.vector.tensor_copy(L[:], ps_r[:])
                    nc.sync.dma_start(
                        out=logits_out[t0 + tt * P: t0 + (tt + 1) * P, :],
                        in_=L[:],
                    )
                    # top-2 + renormalize:  w1 = sigmoid(l1-l2), w2 = 1-w1
                    m1 = rtp.tile([P, 1], F32, name="m1")
                    nc.vector.tensor_reduce(m1[:], L[:], axis=AX.X, op=OP.max)
                    is1 = rtp.tile([P, E], F32, name="is1")
                    nc.vector.tensor_scalar(
                        out=is1[:], in0=L[:], scalar1=m1[:], scalar2=None,
                        op0=OP.is_ge,
                    )
                    big = rtp.tile([P, E], F32, name="big")
                    nc.vector.tensor_scalar_mul(big[:], is1[:], -1e30)
                    Lm = rtp.tile([P, E], F32, name="Lm")
                    nc.vector.tensor_add(Lm[:], L[:], big[:])
                    m2 = rtp.tile([P, 1], F32, name="m2")
                    nc.vector.tensor_reduce(m2[:], Lm[:], axis=AX.X, op=OP.max)
                    d12 = rtp.tile([P, 1], F32, name="d12")
                    nc.vector.tensor_sub(d12[:], m1[:], m2[:])
                    s1 = rtp.tile([P, 1], F32, name="s1")
                    nc.scalar.activation(s1[:], d12[:], AF.Sigmoid)
                    s2 = rtp.tile([P, 1], F32, name="s2")
                    nc.vector.tensor_scalar(
                        out=s2[:], in0=s1[:], scalar1=-1.0, scalar2=1.0,
                        op0=OP.mult, op1=OP.add,
                    )
                    sel2 = rtp.tile([P, E], F32, name="sel2")
                    nc.vector.tensor_scalar(
                        out=sel2[:], in0=L[:], scalar1=m2[:], scalar2=None,
                        op0=OP.is_ge,
                    )
                    is2 = rtp.tile([P, E], F32, name="is2")
                    nc.vector.tensor_sub(is2[:], sel2[:], is1[:])
                    w1 = rtp.tile([P, E], F32, name="w1")
                    nc.vector.tensor_scalar_mul(w1[:], is1[:], s1[:])
                    wr = rtp.tile([P, E], F32, name="wr")
                    nc.vector.scalar_tensor_tensor(
                        out=wr[:], in0=is2[:], scalar=s2[:], in1=w1[:],
                        op0=OP.mult, op1=OP.add,
                    )
                    wsel = rtp.tile([P, E], F32, name="wsel")
                    nc.vector.tensor_mul(wsel[:], wr[:], sel_sb[:])
                    nc.vector.tensor_reduce(
                        rsc[:, g_tt:g_tt + 1], wsel[:], axis=AX.X, op=OP.add,
                    )

                # ---- two half-sweeps over I: SwiGLU then down-proj partial ----
                acc = {}
                for half in range(2):
                    i_lo, i_hi = half * (NI // 2), (half + 1) * (NI // 2)

                    # phase 1: hiddenT tiles [P(i), TB] for this half
                    hid = {}
                    for ii in range(i_lo, i_hi):
                        wg_t = wgp.tile([P, NH * P], F32R, name="wg_t")
                        nc.sync.dma_start(out=wg_t[:], in_=wg[ii, :, :])
                        wi_t = wip.tile([P, NH * P], F32R, name="wi_t")
                        nc.sync.dma_start(out=wi_t[:], in_=wi[ii, :, :])
                        hid_t = hidp.tile([P, TB], F32R, name="hid_t")
                        for tcx in range(TB // NC2):
                            sl = slice(tcx * NC2, (tcx + 1) * NC2)
                            ps_g = psg.tile([P, NC2], F32, name="ps_g")
                            ps_v = psv.tile([P, NC2], F32, name="ps_v")
                            for hh in range(NH):
                                nc.tensor.matmul(
                                    ps_g[:],
                                    lhsT=wg_t[:, hh * P:(hh + 1) * P],
                                    rhs=ft[hh][:, sl],
                                    start=(hh == 0), stop=(hh == NH - 1),
                                )
                            for hh in range(NH):
                                nc.tensor.matmul(
                                    ps_v[:],
                                    lhsT=wi_t[:, hh * P:(hh + 1) * P],
                                    rhs=ft[hh][:, sl],
                                    start=(hh == 0), stop=(hh == NH - 1),
                                )
                            si_t = evp.tile([P, NC2], F32, name="si_t")
                            nc.scalar.activation(si_t[:], ps_g[:], AF.Silu)
                            nc.vector.tensor_mul(hid_t[:, sl], si_t[:], ps_v[:])
                        hid[ii] = hid_t

                    # phase 2: out[t,h] partial over this I-half
                    for ii in range(i_lo, i_hi):
                        wo_t = wop.tile([P, H], F32R, name="wo_t")
                        nc.sync.dma_start(out=wo_t[:], in_=wo[ii, :, :])
                        hid[ii] = (hid[ii], wo_t)

                    for tt in range(NT):
                        g_tt = blk * NT + tt
                        tsl = slice(tt * P, (tt + 1) * P)
                        for hcx in range(H // NC2):
                            hsl = slice(hcx * NC2, (hcx + 1) * NC2)
                            ps_o = pso.tile([P, NC2], F32, name="ps_o")
                            for ii in range(i_lo, i_hi):
                                hid_t, wo_t = hid[ii]
                                nc.tensor.matmul(
                                    ps_o[:],
                                    lhsT=hid_t[:, tsl],
                                    rhs=wo_t[:, hsl],
                                    start=(ii == i_lo), stop=(ii == i_hi - 1),
                                )
                            if half == 0:
                                # stash routing-scaled partial in SBUF
                                a_t = accp.tile([P, NC2], F32, name="a_t")
                                nc.vector.tensor_scalar_mul(
                                    a_t[:], ps_o[:], rsc[:, g_tt:g_tt + 1],
                                )
                                acc[(tt, hcx)] = a_t
                            else:
                                o_t = outp.tile([P, NC2], F32, name="o_t")
                                nc.vector.scalar_tensor_tensor(
                                    out=o_t[:], in0=ps_o[:],
                                    scalar=rsc[:, g_tt:g_tt + 1],
                                    in1=acc[(tt, hcx)][:],
                                    op0=OP.mult, op1=OP.add,
                                )
                                nc.sync.dma_start(
                                    out=out[t0 + tt * P: t0 + (tt + 1) * P, hsl],
                                    in_=o_t[:],
                                )
    nc.finalize()
    return nc


def _build_nc_sparse(blocks):
    """Capacity-gathered variant: expert FFN computed only for this core's
    assigned tokens (host gathers per-expert token sets; zero-weight tokens
    contribute exactly 0 in the reference sum, so skipping them is exact).

    blocks: token-block sizes for the gathered dimension, e.g. [1024, 512]
    for capacity 1536. Full-T router logits are still computed on device
    ([E, T] layout, host transposes); per-gathered-token top-2 weights are
    computed on device from compensated logits.
    """
    import concourse.bass as bass
    import concourse.bacc as bacc
    from concourse import mybir
    from concourse import tile
    from concourse import masks
    from contextlib import ExitStack

    F32 = mybir.dt.float32
    F32R = mybir.dt.float32r
    AF = mybir.ActivationFunctionType
    OP = mybir.AluOpType
    AX = mybir.AxisListType

    CAP = sum(blocks)
    nc = bacc.Bacc(None, target_bir_lowering=False)

    flatT = nc.dram_tensor("flatT", [NH, P, T], F32R, kind="ExternalInput")
    fg = nc.dram_tensor("fg", [NH, P, CAP], F32R, kind="ExternalInput")
    fg_lo = nc.dram_tensor("fg_lo", [NH, P, CAP], F32R, kind="ExternalInput")
    rk_h = nc.dram_tensor("rk_h", [NH, P, E], F32R, kind="ExternalInput")
    rk_l = nc.dram_tensor("rk_l", [NH, P, E], F32R, kind="ExternalInput")
    wg = nc.dram_tensor("wg", [NI, P, NH * P], F32R, kind="ExternalInput")
    wi = nc.dram_tensor("wi", [NI, P, NH * P], F32R, kind="ExternalInput")
    wo = nc.dram_tensor("wo", [NI, P, H], F32R, kind="ExternalInput")
    sel = nc.dram_tensor("sel", [P, E], F32, kind="ExternalInput")
    out = nc.dram_tensor("out", [CAP, H], F32, kind="ExternalOutput")
    logitsT_out = nc.dram_tensor("logitsT", [E, T], F32, kind="ExternalOutput")

    with ExitStack() as st:
        tc = st.enter_context(tile.TileContext(nc))
        pool = lambda name, bufs, **kw: st.enter_context(
            tc.tile_pool(name=name, bufs=bufs, **kw))
        consts = pool("consts", 1)
        flatp = pool("flatp", 9)
        wgp = pool("wgp", 4)
        wip = pool("wip", 4)
        wop = pool("wop", 9)
        hidp = pool("hidp", 9)
        evp = pool("evp", 3)
        accp = pool("accp", 17)
        outp = pool("outp", 3)
        rtp = pool("rtp", 4)
        flop = pool("flop", 3)
        ltp = pool("ltp", 3)
        rfp = pool("rfp", 3)
        psg = pool("psg", 2, space="PSUM")
        psv = pool("psv", 2, space="PSUM")
        pso = pool("pso", 2, space="PSUM")
        psT = pool("psT", 1, space="PSUM")
        psr = pool("psr", 1, space="PSUM")

        rkh_sb = consts.tile([P, NH, E], F32R)
        nc.sync.dma_start(out=rkh_sb[:], in_=rk_h[:].rearrange("h p e -> p h e"))
        rkl_sb = consts.tile([P, NH, E], F32R)
        nc.sync.dma_start(out=rkl_sb[:], in_=rk_l[:].rearrange("h p e -> p h e"))
        ident = consts.tile([P, P], F32)
        masks.make_identity(nc, ident[:])
        sel_sb = consts.tile([P, E], F32)
        nc.sync.dma_start(out=sel_sb[:], in_=sel[:])
        rsc = consts.tile([P, 32], F32)

        preloaded = {}

        # full-T router logits chunks, interleaved into the expert schedule
        rf_remaining = list(range(T // NC2))

        def emit_rf_chunk():
            if not rf_remaining:
                return
            tcx = rf_remaining.pop(0)
            ps_lt = psT.tile([E, NC2], F32, name="ps_lt")
            for hh in range(NH):
                rf = rfp.tile([P, NC2], F32R, name="rf")
                nc.sync.dma_start(
                    out=rf[:],
                    in_=flatT[hh, :, tcx * NC2:(tcx + 1) * NC2])
                nc.tensor.matmul(
                    ps_lt[:], lhsT=rkh_sb[:, hh, :], rhs=rf[:],
                    start=(hh == 0), stop=(hh == NH - 1))
            lt_sb = ltp.tile([E, NC2], F32, name="lt_sb")
            nc.vector.tensor_copy(lt_sb[:], ps_lt[:])
            nc.sync.dma_start(
                out=logitsT_out[:, tcx * NC2:(tcx + 1) * NC2], in_=lt_sb[:])

        t0g = 0
        for blk, TBg in enumerate(blocks):
            NTg = TBg // P

            # load activations in 512-token chunks so the router can start
            # after the first chunk column instead of the full block
            ft = []
            for hh in range(NH):
                f_t = flatp.tile([P, TB], F32R, name="ft")
                ft.append(f_t)
            chunks = [(o, min(NC2, TBg - o)) for o in range(0, TBg, NC2)]
            for (co, cw) in chunks:
                for hh in range(NH):
                    nc.sync.dma_start(
                        out=ft[hh][:, co:co + cw],
                        in_=fg[hh, :, t0g + co:t0g + co + cw])
                if blk == 0 and co == 0:
                    # prefetch the first expert weight tiles behind the first
                    # activation chunk so phase 1 starts right after the
                    # router without delaying the router itself
                    for ii in range(2):
                        pwg = wgp.tile([P, NH * P], F32R, name="wg_t")
                        nc.sync.dma_start(out=pwg[:], in_=wg[ii, :, :])
                        pwi = wip.tile([P, NH * P], F32R, name="wi_t")
                        nc.sync.dma_start(out=pwi[:], in_=wi[ii, :, :])
                        preloaded[ii] = (pwg, pwi)

            # gathered-token router: compensated logits -> top-2 weights
            lts = []
            for (co, cw) in chunks:
                sl = slice(co, co + cw)
                ps_lt = psT.tile([E, NC2], F32, name="ps_lt")
                for hh in range(NH):
                    fl = flop.tile([P, NC2], F32R, name="fl")
                    nc.sync.dma_start(
                        out=fl[:, :cw],
                        in_=fg_lo[hh, :, t0g + co:t0g + co + cw])
                    nc.tensor.matmul(
                        ps_lt[:, :cw], lhsT=rkh_sb[:, hh, :],
                        rhs=ft[hh][:, sl], start=(hh == 0), stop=False)
                    nc.tensor.matmul(
                        ps_lt[:, :cw], lhsT=rkl_sb[:, hh, :],
                        rhs=ft[hh][:, sl], start=False, stop=False)
                    nc.tensor.matmul(
                        ps_lt[:, :cw], lhsT=rkh_sb[:, hh, :], rhs=fl[:, :cw],
                        start=False, stop=(hh == NH - 1))
                lt_sb = ltp.tile([E, NC2], F32, name="lt_sb")
                nc.vector.tensor_copy(lt_sb[:, :cw], ps_lt[:, :cw])
                lts.append(lt_sb)

            for tt in range(NTg):
                g_tt = (t0g + tt * P) // P
                tcx, off = divmod(tt * P, NC2)
                ps_r = psr.tile([P, E], F32, name="ps_r")
                nc.tensor.transpose(
                    ps_r[:], lts[tcx][:, off:off + P], ident[:E, :E])
                L = rtp.tile([P, E], F32, name="L")
                nc.vector.tensor_copy(L[:], ps_r[:])
                m1 = rtp.tile([P, 1], F32, name="m1")
                nc.vector.tensor_reduce(m1[:], L[:], axis=AX.X, op=OP.max)
                is1 = rtp.tile([P, E], F32, name="is1")
                nc.vector.tensor_scalar(
                    out=is1[:], in0=L[:], scalar1=m1[:], scalar2=None,
                    op0=OP.is_ge)
                big = rtp.tile([P, E], F32, name="big")
                nc.vector.tensor_scalar_mul(big[:], is1[:], -1e30)
                Lm = rtp.tile([P, E], F32, name="Lm")
                nc.vector.tensor_add(Lm[:], L[:], big[:])
                m2 = rtp.tile([P, 1], F32, name="m2")
                nc.vector.tensor_reduce(m2[:], Lm[:], axis=AX.X, op=OP.max)
                d12 = rtp.tile([P, 1], F32, name="d12")
                nc.vector.tensor_sub(d12[:], m1[:], m2[:])
                s1 = rtp.tile([P, 1], F32, name="s1")
                nc.scalar.activation(s1[:], d12[:], AF.Sigmoid)
                s2 = rtp.tile([P, 1], F32, name="s2")
                nc.vector.tensor_scalar(
                    out=s2[:], in0=s1[:], scalar1=-1.0, scalar2=1.0,
                    op0=OP.mult, op1=OP.add)
                sel2 = rtp.tile([P, E], F32, name="sel2")
                nc.vector.tensor_scalar(
                    out=sel2[:], in0=L[:], scalar1=m2[:], scalar2=None,
                    op0=OP.is_ge)
                is2 = rtp.tile([P, E], F32, name="is2")
                nc.vector.tensor_sub(is2[:], sel2[:], is1[:])
                w1 = rtp.tile([P, E], F32, name="w1")
                nc.vector.tensor_scalar_mul(w1[:], is1[:], s1[:])
                wr = rtp.tile([P, E], F32, name="wr")
                nc.vector.scalar_tensor_tensor(
                    out=wr[:], in0=is2[:], scalar=s2[:], in1=w1[:],
                    op0=OP.mult, op1=OP.add)
                wsel = rtp.tile([P, E], F32, name="wsel")
                nc.vector.tensor_mul(wsel[:], wr[:], sel_sb[:])
                nc.vector.tensor_reduce(
                    rsc[:, g_tt:g_tt + 1], wsel[:], axis=AX.X, op=OP.add)

            emit_rf_chunk()

            acc = {}
            for half in range(2):
                i_lo, i_hi = half * (NI // 2), (half + 1) * (NI // 2)

                hid = {}
                for ii in range(i_lo, i_hi):
                    if blk == 0 and ii in (2, 5, 10, 13):
                        emit_rf_chunk()
                    if blk == 0 and half == 0 and ii in preloaded:
                        wg_t, wi_t = preloaded[ii]
                    else:
                        wg_t = wgp.tile([P, NH * P], F32R, name="wg_t")
                        nc.sync.dma_start(out=wg_t[:], in_=wg[ii, :, :])
                        wi_t = wip.tile([P, NH * P], F32R, name="wi_t")
                        nc.sync.dma_start(out=wi_t[:], in_=wi[ii, :, :])
                    hid_t = hidp.tile([P, TB], F32R, name="hid_t")
                    for (co, cw) in chunks:
                        sl = slice(co, co + cw)
                        ps_g = psg.tile([P, NC2], F32, name="ps_g")
                        ps_v = psv.tile([P, NC2], F32, name="ps_v")
                        for hh in range(NH):
                            nc.tensor.matmul(
                                ps_g[:, :cw],
                                lhsT=wg_t[:, hh * P:(hh + 1) * P],
                                rhs=ft[hh][:, sl],
                                start=(hh == 0), stop=(hh == NH - 1))
                        for hh in range(NH):
                            nc.tensor.matmul(
                                ps_v[:, :cw],
                                lhsT=wi_t[:, hh * P:(hh + 1) * P],
                                rhs=ft[hh][:, sl],
                                start=(hh == 0), stop=(hh == NH - 1))
                        si_t = evp.tile([P, NC2], F32, name="si_t")
                        nc.scalar.activation(si_t[:, :cw], ps_g[:, :cw], AF.Silu)
                        nc.vector.tensor_mul(
                            hid_t[:, sl], si_t[:, :cw], ps_v[:, :cw])
                    hid[ii] = hid_t

                for ii in range(i_lo, i_hi):
                    wo_t = wop.tile([P, H], F32R, name="wo_t")
                    nc.sync.dma_start(out=wo_t[:], in_=wo[ii, :, :])
                    hid[ii] = (hid[ii], wo_t)

                for tt in range(NTg):
                    g_tt = (t0g + tt * P) // P
                    tsl = slice(tt * P, (tt + 1) * P)
                    for hcx in range(H // NC2):
                        hsl = slice(hcx * NC2, (hcx + 1) * NC2)
                        ps_o = pso.tile([P, NC2], F32, name="ps_o")
                        for ii in range(i_lo, i_hi):
                            hid_t, wo_t = hid[ii]
                            nc.tensor.matmul(
                                ps_o[:],
                                lhsT=hid_t[:, tsl],
                                rhs=wo_t[:, hsl],
                                start=(ii == i_lo), stop=(ii == i_hi - 1))
                        if half == 0:
                            a_t = accp.tile([P, NC2], F32, name="a_t")
                            nc.vector.tensor_scalar_mul(
                                a_t[:], ps_o[:], rsc[:, g_tt:g_tt + 1])
                            acc[(tt, hcx)] = a_t
                        else:
                            o_t = outp.tile([P, NC2], F32, name="o_t")
                            nc.vector.scalar_tensor_tensor(
                                out=o_t[:], in0=ps_o[:],
                                scalar=rsc[:, g_tt:g_tt + 1],
                                in1=acc[(tt, hcx)][:],
                                op0=OP.mult, op1=OP.add)
                            nc.sync.dma_start(
                                out=out[t0g + tt * P: t0g + (tt + 1) * P,
                                        hsl],
                                in_=o_t[:])
                emit_rf_chunk()
            t0g += TBg
        while rf_remaining:
            emit_rf_chunk()
    nc.finalize()
    return nc


def _fp22_split(x):
    """x -> (hi, lo): hi = x truncated to 13 mantissa bits (exact FP22),
    lo = x - hi (exact in fp32)."""
    hi = (x.view(np.uint32) & np.uint32(0xFFFFE000)).view(np.float32)
    return hi, x - hi


def _prep_in_maps(hidden_states, router_kernel, w_gate, w_inter, w_out):
    f32 = np.float32
    flat = np.ascontiguousarray(hidden_states, dtype=f32).reshape(T, H)
    flatT_full = np.ascontiguousarray(flat.T)
    flatT, flatT_lo = _fp22_split(flatT_full)
    flatT = flatT.reshape(NH, P, T)
    flatT_lo = np.ascontiguousarray(flatT_lo).reshape(NH, P, T)
    rk = np.ascontiguousarray(router_kernel, dtype=f32)
    rk_h, rk_l = _fp22_split(rk)
    rk_h = rk_h.reshape(NH, P, E)
    rk_l = np.ascontiguousarray(rk_l).reshape(NH, P, E)
    in_maps = []
    for c in range(NCORES):
        wg_c = np.ascontiguousarray(
            np.asarray(w_gate[c], dtype=f32).reshape(NH, P, NI, P)
            .transpose(2, 1, 0, 3).reshape(NI, P, NH * P))
        wi_c = np.ascontiguousarray(
            np.asarray(w_inter[c], dtype=f32).reshape(NH, P, NI, P)
            .transpose(2, 1, 0, 3).reshape(NI, P, NH * P))
        wo_c = np.ascontiguousarray(
            np.asarray(w_out[c], dtype=f32).reshape(NI, P, H))
        sel_c = np.zeros((P, E), dtype=f32)
        sel_c[:, c] = 1.0
        in_maps.append({
            "flatT": flatT, "flatT_lo": flatT_lo, "rk_h": rk_h, "rk_l": rk_l,
            "wg": wg_c, "wi": wi_c, "wo": wo_c, "sel": sel_c,
        })
    return in_maps


def _expert_weight_maps(w_gate, w_inter, w_out):
    f32 = np.float32
    maps = []
    for c in range(NCORES):
        wg_c = np.ascontiguousarray(
            np.asarray(w_gate[c], dtype=f32).reshape(NH, P, NI, P)
            .transpose(2, 1, 0, 3).reshape(NI, P, NH * P))
        wi_c = np.ascontiguousarray(
            np.asarray(w_inter[c], dtype=f32).reshape(NH, P, NI, P)
            .transpose(2, 1, 0, 3).reshape(NI, P, NH * P))
        wo_c = np.ascontiguousarray(
            np.asarray(w_out[c], dtype=f32).reshape(NI, P, H))
        sel_c = np.zeros((P, E), dtype=f32)
        sel_c[:, c] = 1.0
        maps.append({"wg": wg_c, "wi": wi_c, "wo": wo_c, "sel": sel_c})
    return maps


def run_on_device(hidden_states, router_kernel, w_gate, w_inter, w_out,
                  trace=False, force_dense=False, **trace_kw):
    """Shard + run SPMD on 8 cores; returns (out, logits, results)."""
    from concourse.bass_utils import run_bass_kernel_spmd

    f32 = np.float32
    flat = np.ascontiguousarray(hidden_states, dtype=f32).reshape(T, H)
    rk = np.ascontiguousarray(router_kernel, dtype=f32)
    rk_h, rk_l = _fp22_split(rk)
    rk_h = rk_h.reshape(NH, P, E)
    rk_l = np.ascontiguousarray(rk_l).reshape(NH, P, E)

    # host-side dispatch (sharding decision): exact top-2 per token
    logits64 = flat.astype(np.float64) @ rk.astype(np.float64)
    srt = np.sort(logits64, -1)
    margin = (srt[:, -2] - srt[:, -3]).min()
    thr = srt[:, -2:-1]                      # 2nd-largest logit per token
    sel_mask = logits64 >= thr               # [T, E] top-2 membership
    counts = sel_mask.sum(0)
    cap_raw = int(counts.max())
    use_sparse = (not force_dense) and margin > 1e-5 and cap_raw <= 3584

    wmaps = _expert_weight_maps(w_gate, w_inter, w_out)

    if use_sparse:
        cap = max(512, ((cap_raw + 255) // 256) * 256)
        blocks = [1024] * (cap // 1024)
        if cap % 1024:
            blocks.append(cap % 1024)
        key = ("sparse", cap)
        if key not in _CACHE:
            _CACHE[key] = _build_nc_sparse(blocks)
        nc = _CACHE[key]

        flatT_full = np.ascontiguousarray(flat.T).reshape(NH, P, T)
        idxs, in_maps = [], []
        for c in range(NCORES):
            idx = np.nonzero(sel_mask[:, c])[0]
            idxs.append(idx)
            gath = np.zeros((cap, H), f32)
            gath[:len(idx)] = flat[idx]
            gT = np.ascontiguousarray(gath.T)
            g_hi, g_lo = _fp22_split(gT)
            in_maps.append({
                "flatT": flatT_full,
                "fg": g_hi.reshape(NH, P, cap),
                "fg_lo": np.ascontiguousarray(g_lo).reshape(NH, P, cap),
                "rk_h": rk_h, "rk_l": rk_l, **wmaps[c],
            })
        res = run_bass_kernel_spmd(
            nc, in_maps, core_ids=list(range(NCORES)), trace=trace,
            **trace_kw)
        parts = res.results
        out = np.zeros((T, H), f32)
        for c in range(NCORES):
            n = len(idxs[c])
            out[idxs[c]] += parts[c]["out"][:n]
        logits = np.ascontiguousarray(parts[0]["logitsT"].T)
    else:
        key = ("dense",)
        if key not in _CACHE:
            _CACHE[key] = _build_nc()
        nc = _CACHE[key]
        flatT_full, flatT_lo = _fp22_split(np.ascontiguousarray(flat.T))
        in_maps = []
        for c in range(NCORES):
            in_maps.append({
                "flatT": flatT_full.reshape(NH, P, T),
                "flatT_lo": np.ascontiguousarray(flatT_lo).reshape(NH, P, T),
                "rk_h": rk_h, "rk_l": rk_l, **wmaps[c],
            })
        res = run_bass_kernel_spmd(
            nc, in_maps, core_ids=list(range(NCORES)), trace=trace,
            **trace_kw)
        parts = res.results
        out = parts[0]["out"].astype(np.float32, copy=True)
        for c in range(1, NCORES):
            out += parts[c]["out"]
        logits = parts[0]["logits"]
    return out.reshape(2, 2048, H), logits, res


def kernel(hidden_states, router_kernel, w_gate, w_inter, w_out):
    out, logits, _ = run_on_device(
        hidden_states, router_kernel, w_gate, w_inter, w_out)
    return out, logits
